# revision 12
# baseline (speedup 1.0000x reference)
"""Trainium2 Bass kernel for a single-step attention GRU decoder.

Math (per batch row b):
    A        = hidden @ W_hp + b_hp + b_ep                  [B, H]
    enc_attn = enc[t,b,:] @ W_ep                            [T, B, H]
    scores   = tanh(enc_attn + A[b]) @ w_v                  [B, T]
    attnw    = softmax(scores, axis=t)                      [B, T]   (output 3)
    context  = sum_t attnw[b,t] * enc[t,b,:]                [B, H]
    cs       = context @ W_cs + b_cs                        [B, 250]
    in_dec   = concat(emb[argmax(in_char)], cs)             [B, 300]
    GRU(in_dec, hidden) -> new_h                            [B, H]   (output 2)
    probs    = softmax(new_h @ W_out + b_out)               [B, 83]  (output 1)

Sharding: data-parallel over B=256 across 8 cores (32 rows each); all
weights replicated.  Inside a core everything runs in a "transposed"
layout (feature dim on partitions, batch on the free dim) so weight
matrices are stationary matmul operands in their natural [K_in, M_out]
layout and per-feature biases are per-partition scalars fused into
ScalarE activations.

enc (39 MB/core) is DMAed in natural layout [t, h] per batch row,
transposed on the PE (128x128 identity matmuls) to get enc^T [h, t] for
the score matmul; the natural copy is reused as the moving operand of
the context matmul (stationary = attnw^T column, contraction over t).
The big score matmul runs in float32r (single-pass fp32, 1 cycle/row at
N>=256 vs 4 for full fp32).

Engine APs can only start at partitions {0,32,64,96}, so per-row work
is placed at those bases (4 batch rows per "quad"), single rows move
between partitions via SBUF<->SBUF DMA, and the attention softmax is
only computed up to exp() on device - the 1/sum normalisation of the
attnw output happens on the host while the context matmul folds the
normalisation into its PSUM->SBUF eviction (ScalarE scale).
"""

import numpy as np

import concourse.bass as bass
import concourse.bacc as bacc
import concourse.mybir as mybir
import concourse.tile as tile
from concourse import bass_utils, masks

dt = mybir.dt
F32 = dt.float32
F32R = dt.float32r
AF = mybir.ActivationFunctionType
ALU = mybir.AluOpType
AX = mybir.AxisListType

N_CORES = 8
B = 32          # per-core batch
T = 600
H = 512
V = 83
EMB = 50
CS = 250        # 5*EMB
GI = 300        # 6*EMB
G3 = 3 * H      # 1536

TCH = [(0, 128), (128, 128), (256, 128), (384, 128), (512, 88)]
NK = H // 128   # 4 h chunks

# vecbank columns
VB_WV = 0       # 4: w_v chunks
VB_AB = 4       # 4: b_hp + b_ep chunks
VB_RZ = 8       # 8: (b_ih + b_hh)[0:1024] chunks
VB_IN = 16      # 4: b_ih[1024:1536] chunks
VB_HN = 20      # 4: b_hh[1024:1536] chunks
VB_CS = 24      # 2: b_cs chunks (128, 122)
VB_BO = 26      # 1: b_out (83)
VB_N = 27

# in_dec^T partition chunks: [emb(50) | cs 0:128 | cs 128:250]
GIK = [EMB, 128, CS - 128]          # 50, 128, 122
CS_SPLIT = [(0, 128, 1), (128, CS - 128, 2)]   # (cs_off, n_rows, chunk_idx)


def _r(ap):
    return ap.bitcast(F32R)


def build_program():
    nc = bacc.Bacc("TRN2", target_bir_lowering=False, debug=False,
                   num_devices=N_CORES)

    din = {}
    def inp(name, shape):
        din[name] = nc.dram_tensor(name, list(shape), F32, kind="ExternalInput")
        return din[name]

    inp("in_char", (B, V))
    inp("hidden", (B, H))
    din["enc"] = nc.dram_tensor("enc", [T, B, H], F32R, kind="ExternalInput")
    inp("W_hp", (H, H))
    din["W_ep"] = nc.dram_tensor("W_ep", [H, H], F32R, kind="ExternalInput")
    din["wv_bank"] = nc.dram_tensor("wv_bank", [128, NK], F32R, kind="ExternalInput")
    din["ident_r"] = nc.dram_tensor("ident_r", [128, 128], F32R, kind="ExternalInput")
    inp("W_cs", (H, CS))
    inp("emb", (V, EMB))
    inp("W_ih", (GI, G3))        # device re-chunks rows to (50,128,122)
    inp("W_hh", (H, G3))
    inp("W_out", (H, V))
    inp("vecbank", (128, VB_N))

    d_probs = nc.dram_tensor("out_probs", [B, V], F32, kind="ExternalOutput")
    d_newh = nc.dram_tensor("out_newh", [B, H], F32, kind="ExternalOutput")
    d_expw = nc.dram_tensor("out_expw", [B, T], F32, kind="ExternalOutput")
    d_rinv = nc.dram_tensor("out_rinv", [1, B], F32, kind="ExternalOutput")

    with tile.TileContext(nc) as tc:
        with (
            tc.tile_pool(name="const", bufs=1) as cpool,
            tc.tile_pool(name="enc", bufs=6) as encpool,
            tc.tile_pool(name="work", bufs=2) as wpool,
            tc.tile_pool(name="psum", bufs=1, space=bass.MemorySpace.PSUM) as ppool,
        ):
            build_body(nc, tc, cpool, encpool, wpool, ppool, din,
                       d_probs, d_newh, d_expw, d_rinv)

    nc.compile()
    return nc


def build_body(nc, tc, cpool, encpool, wpool, ppool, din,
               d_probs, d_newh, d_expw, d_rinv):
    sync = nc.sync

    # ---------------- constants into SBUF ----------------
    Wep_sb = cpool.tile([128, NK, H], F32R)
    sync.dma_start(Wep_sb[:], din["W_ep"].ap().rearrange("(k p) n -> p k n", p=128))
    Whp_sb = cpool.tile([128, NK, H], F32)
    sync.dma_start(Whp_sb[:], din["W_hp"].ap().rearrange("(k p) n -> p k n", p=128))
    Wcs_sb = cpool.tile([128, NK, CS], F32)
    sync.dma_start(Wcs_sb[:], din["W_cs"].ap().rearrange("(k p) n -> p k n", p=128))
    Whh_sb = cpool.tile([128, NK, G3], F32)
    sync.dma_start(Whh_sb[:], din["W_hh"].ap().rearrange("(k p) n -> p k n", p=128))
    Wout_sb = cpool.tile([128, NK, V], F32)
    sync.dma_start(Wout_sb[:], din["W_out"].ap().rearrange("(k p) n -> p k n", p=128))
    Wih_sb = cpool.tile([128, 3, G3], F32)
    sync.dma_start(Wih_sb[0:GIK[0], 0, :], din["W_ih"][0:50, :])
    sync.dma_start(Wih_sb[0:GIK[1], 1, :], din["W_ih"][50:178, :])
    sync.dma_start(Wih_sb[0:GIK[2], 2, :], din["W_ih"][178:300, :])
    emb_sb = cpool.tile([V, EMB], F32)
    sync.dma_start(emb_sb[:], din["emb"].ap())
    vb = cpool.tile([128, VB_N], F32)
    sync.dma_start(vb[:], din["vecbank"].ap())
    wv_sb = cpool.tile([128, NK], F32R)
    sync.dma_start(wv_sb[:], din["wv_bank"].ap())
    ic_sb = cpool.tile([B, V], F32)
    sync.dma_start(ic_sb[:], din["in_char"].ap())
    hid_sb = cpool.tile([B, H], F32)
    sync.dma_start(hid_sb[:], din["hidden"].ap())

    ident = cpool.tile([128, 128], F32)
    masks.make_identity(nc, ident[:])
    ident_r = cpool.tile([128, 128], F32R)
    sync.dma_start(ident_r[:], din["ident_r"].ap())
    ones_sb = cpool.tile([128, 1], F32)
    nc.gpsimd.memset(ones_sb[:], 1.0)

    # persistent per-core intermediates
    AT_sb = cpool.tile([128, NK, B], F32)        # (hid @ W_hp + b_hp + b_ep)^T
    hT_sb = cpool.tile([128, NK, B], F32)
    rinv_all = cpool.tile([1, B], F32)           # 1/sum(exp(scores)) per row
    ctx_nat = cpool.tile([B, H], F32)            # normalised context rows
    ctxT_sb = cpool.tile([128, NK, B], F32)
    indec_sb = cpool.tile([128, 3, B], F32)
    rz_sb = cpool.tile([128, 8, B], F32)         # r gates 0:4, z gates 4:8
    n_sb = cpool.tile([128, NK, B], F32)
    newhT_sb = cpool.tile([128, NK, B], F32)
    newh_nat = cpool.tile([B, H], F32)
    lgT_sb = cpool.tile([V, B], F32)
    lg_nat = cpool.tile([B, V], F32)
    expv = cpool.tile([B, V], F32)
    probs_sb = cpool.tile([B, V], F32)
    smax = cpool.tile([B, 1], F32)
    snegmax = cpool.tile([B, 1], F32)
    ssum = cpool.tile([B, 1], F32)
    srecip = cpool.tile([B, 1], F32)

    def transpose_f32(psum_out, in_ap, n_rows):
        # psum_out <- in_ap.T ; in_ap is [n_rows, M] at partition base 0
        nc.tensor.transpose(psum_out, in_ap, ident[0:n_rows, 0:n_rows])

    def transpose_f32r(psum_out, in_ap, n_rows):
        nc.tensor.transpose(psum_out, in_ap, ident_r[0:n_rows, 0:n_rows])

    # ---------------- stage A: hT, A^T, embedding ----------------
    for k in range(NK):
        p_tr = ppool.tile([128, 512], F32, tag="ptr", bufs=2, name=f"p_hT{k}")
        transpose_f32(p_tr[:, 0:B], hid_sb[:, 128 * k:128 * (k + 1)], B)
        nc.vector.tensor_copy(hT_sb[:, k, :], p_tr[:, 0:B])

    for j in range(NK):
        p_a = ppool.tile([128, B], F32, tag="pawt", bufs=1, name=f"p_A{j}")
        for k in range(NK):
            nc.tensor.matmul(p_a[:], Whp_sb[:, k, 128 * j:128 * (j + 1)],
                             hT_sb[:, k, :], start=(k == 0), stop=(k == NK - 1))
        nc.scalar.activation(AT_sb[:, j, :], p_a[:], AF.Identity,
                             bias=vb[:, VB_AB + j:VB_AB + j + 1])

    # argmax(in_char) -> one-hot -> emb rows into in_dec^T chunk 0
    icmax = cpool.tile([B, 1], F32)
    onehot = cpool.tile([B, V], F32)
    onehotT = cpool.tile([V, B], F32)
    nc.vector.tensor_reduce(icmax[:], ic_sb[:], axis=AX.X, op=ALU.max)
    nc.vector.tensor_scalar(onehot[:], ic_sb[:], icmax[:], None, op0=ALU.is_ge)
    p_oh = ppool.tile([128, B], F32, tag="pawt", bufs=1, name="p_oh")
    transpose_f32(p_oh[0:V, :], onehot[:], B)
    nc.vector.tensor_copy(onehotT[:], p_oh[0:V, :])
    p_et = ppool.tile([128, B], F32, tag="pawt", bufs=1, name="p_et")
    nc.tensor.matmul(p_et[0:EMB, :], emb_sb[:], onehotT[:], start=True, stop=True)
    nc.scalar.copy(indec_sb[0:EMB, 0, :], p_et[0:EMB, :])

    # ---------------- stage B: per-quad attention pipeline ----------------
    for q in range(B // 4):
        enc_nats = []
        expw_p0s = []
        for bi in range(4):
            b = 4 * q + bi
            # natural enc_b: [128, 5, 512]; row p of chunk j is t = 128j + p
            enc_nat = encpool.tile([128, len(TCH), H], F32R, tag="enc_nat",
                                   name=f"enc_nat{b}")
            enc_nats.append(enc_nat)
            sync.dma_start(
                enc_nat[:, 0:4, :],
                din["enc"][0:512, b, :].rearrange("(k p) c -> p k c", p=128))
            sync.dma_start(enc_nat[0:88, 4, :], din["enc"][512:600, b, :])

            # enc^T via PE transposes
            encT = wpool.tile([128, NK, T], F32R, tag="encT", name=f"encT{b}")
            for k in range(NK):
                p_tr = ppool.tile([128, 512], F32R, tag="ptr", bufs=2,
                                  name=f"p_tr{b}_{k}")
                for (ti, (t0, tw)) in enumerate(TCH[:4]):
                    transpose_f32r(p_tr[:, 128 * ti:128 * ti + tw],
                                   enc_nat[0:tw, ti, 128 * k:128 * (k + 1)], tw)
                p_tr2 = ppool.tile([128, 512], F32R, tag="ptr", bufs=2,
                                   name=f"p_tr2{b}_{k}")
                transpose_f32r(p_tr2[:, 0:88],
                               enc_nat[0:88, 4, 128 * k:128 * (k + 1)], 88)
                if k % 2 == 0:
                    nc.vector.tensor_copy(encT[:, k, 0:512], p_tr[:])
                    nc.vector.tensor_copy(encT[:, k, 512:600], p_tr2[:, 0:88])
                else:
                    nc.scalar.copy(encT[:, k, 0:512], p_tr[:])
                    nc.scalar.copy(encT[:, k, 512:600], p_tr2[:, 0:88])

            # enc_attn^T[j] = W_ep[:,j].T @ enc^T ; then tanh(+A bias)
            tanhT = wpool.tile([128, NK, T], F32R, tag="tanhT", name=f"tanhT{b}")
            for j in range(NK):
                for (t0, tw) in ((0, 300), (300, 300)):
                    p_m = ppool.tile([128, 300], F32, tag="pmain", bufs=2,
                                     name=f"p_m{b}_{j}_{t0}")
                    for k in range(NK):
                        nc.tensor.matmul(
                            p_m[:],
                            Wep_sb[:, k, 128 * j:128 * (j + 1)],
                            encT[:, k, t0:t0 + tw],
                            start=(k == 0), stop=(k == NK - 1))
                    nc.scalar.activation(tanhT[:, j, t0:t0 + tw], p_m[:], AF.Tanh,
                                         bias=AT_sb[:, j, b:b + 1])

            # scores row (partition 0): w_v^T @ tanh^T
            qscA = ppool.tile([1, 512], F32, tag="qscA", bufs=1, name=f"qscA{b}")
            qscB = ppool.tile([1, 512], F32, tag="qscB", bufs=1, name=f"qscB{b}")
            for (qsc, t0) in ((qscA, 0), (qscB, 300)):
                for j in range(NK):
                    nc.tensor.matmul(
                        qsc[0:1, 0:300],
                        wv_sb[:, j:j + 1],
                        tanhT[:, j, t0:t0 + 300],
                        start=(j == 0), stop=(j == NK - 1))

            # softmax pieces at partition 0
            # sred cols: 0,1 = halves' max, 2 = row max, 3 = -max
            sred = wpool.tile([1, 4], F32, tag="sred", name=f"sred{b}")
            nc.vector.tensor_reduce(sred[0:1, 0:1], qscA[0:1, 0:300],
                                    axis=AX.X, op=ALU.max)
            nc.vector.tensor_reduce(sred[0:1, 1:2], qscB[0:1, 0:300],
                                    axis=AX.X, op=ALU.max)
            nc.vector.tensor_reduce(sred[0:1, 2:3], sred[0:1, 0:2],
                                    axis=AX.X, op=ALU.max)
            nc.vector.tensor_scalar_mul(sred[0:1, 3:4], sred[0:1, 2:3], -1.0)
            expw_p0 = wpool.tile([1, T], F32, tag="expw_p0", bufs=3,
                                 name=f"expw_p0{b}")
            expw_p0s.append(expw_p0)
            ssum2 = wpool.tile([1, 2], F32, tag="ssum2", name=f"ssum2{b}")
            nc.scalar.activation(expw_p0[0:1, 0:300], qscA[0:1, 0:300], AF.Exp,
                                 bias=sred[0:1, 3:4], accum_out=ssum2[0:1, 0:1])
            nc.scalar.activation(expw_p0[0:1, 300:600], qscB[0:1, 0:300], AF.Exp,
                                 bias=sred[0:1, 3:4], accum_out=ssum2[0:1, 1:2])
            stot = wpool.tile([1, 1], F32, tag="stot", name=f"stot{b}")
            nc.vector.tensor_reduce(stot[0:1, :], ssum2[0:1, 0:2],
                                    axis=AX.X, op=ALU.add)
            nc.vector.reciprocal(rinv_all[0:1, b:b + 1], stot[0:1, :])
            # unnormalised attention row -> DRAM (host divides by the sum)
            sync.dma_start(d_expw[b:b + 1, :], expw_p0[0:1, :])

        # attnw^T columns via ones outer-product (partition 0 only)
        awT = wpool.tile([128, len(TCH), 4], F32R, tag="awT", name=f"awT{q}")
        p_awt = ppool.tile([128, 32], F32, tag="pawt", bufs=1, name=f"p_awt{q}")
        for bi in range(4):
            for (ti, (t0, tw)) in enumerate(TCH):
                nc.tensor.matmul(p_awt[0:tw, 4 * ti + bi:4 * ti + bi + 1],
                                 expw_p0s[bi][0:1, t0:t0 + tw],
                                 ones_sb[0:1, 0:1],
                                 start=True, stop=True)
        for (ti, (t0, tw)) in enumerate(TCH):
            nc.vector.tensor_copy(awT[0:tw, ti, :],
                                  p_awt[0:tw, 4 * ti:4 * ti + 4])

        # context rows: ctx[b] = rinv * sum_t expw[b,t] * enc[t,b,:]
        for bi in range(4):
            b = 4 * q + bi
            pctx = ppool.tile([1, 512], F32, tag="pctx", bufs=1, name=f"pctx{b}")
            for (ti, (t0, tw)) in enumerate(TCH):
                nc.tensor.matmul(pctx[0:1, :],
                                 awT[0:tw, ti, bi:bi + 1],
                                 enc_nats[bi][0:tw, ti, :],
                                 start=(ti == 0), stop=(ti == len(TCH) - 1))
            ctx_p0 = wpool.tile([1, H], F32, tag="ctx_p0", bufs=2,
                                name=f"ctx_p0{b}")
            nc.scalar.activation(ctx_p0[0:1, :], pctx[0:1, :],
                                 AF.Copy, scale=rinv_all[0:1, b:b + 1])
            # move the row into place (DMA crosses partitions freely)
            sync.dma_start(ctx_nat[b:b + 1, :], ctx_p0[0:1, :])

    # ---------------- stage C: epilogue ----------------
    for k in range(NK):
        p_tr = ppool.tile([128, 512], F32, tag="ptr", bufs=2, name=f"p_ctxT{k}")
        transpose_f32(p_tr[:, 0:B], ctx_nat[:, 128 * k:128 * (k + 1)], B)
        nc.vector.tensor_copy(ctxT_sb[:, k, :], p_tr[:, 0:B])

    # context shrink into in_dec^T chunks 1,2
    for (cs0, n_rows, ci) in CS_SPLIT:
        p_cs = ppool.tile([128, B], F32, tag="pawt", bufs=1, name=f"p_cs{ci}")
        for k in range(NK):
            nc.tensor.matmul(p_cs[0:n_rows, :],
                             Wcs_sb[:, k, cs0:cs0 + n_rows],
                             ctxT_sb[:, k, :],
                             start=(k == 0), stop=(k == NK - 1))
        nc.scalar.activation(indec_sb[0:n_rows, ci, :],
                             p_cs[0:n_rows, :], AF.Identity,
                             bias=vb[0:n_rows, VB_CS + ci - 1:VB_CS + ci])

    # GRU gates, chunks of 128 over 3H
    def gate_matmuls(p_g, c, with_ih, with_hh):
        nmm = (3 if with_ih else 0) + (NK if with_hh else 0)
        i = 0
        if with_ih:
            for k in range(3):
                kw = GIK[k]
                nc.tensor.matmul(p_g[:], Wih_sb[0:kw, k, 128 * c:128 * (c + 1)],
                                 indec_sb[0:kw, k, :],
                                 start=(i == 0), stop=(i == nmm - 1))
                i += 1
        if with_hh:
            for k in range(NK):
                nc.tensor.matmul(p_g[:], Whh_sb[:, k, 128 * c:128 * (c + 1)],
                                 hT_sb[:, k, :],
                                 start=(i == 0), stop=(i == nmm - 1))
                i += 1

    for c in range(8):  # r and z gates
        p_g = ppool.tile([128, B], F32, tag="pawt", bufs=1, name=f"p_g{c}")
        gate_matmuls(p_g, c, True, True)
        nc.scalar.activation(rz_sb[:, c, :], p_g[:], AF.Sigmoid,
                             bias=vb[:, VB_RZ + c:VB_RZ + c + 1])

    hn_sb = cpool.tile([128, B], F32)
    rhn_sb = cpool.tile([128, B], F32)
    gin_sb = cpool.tile([128, B], F32)
    for k in range(NK):  # n gate chunks + new_h
        c = 8 + k
        p_gh = ppool.tile([128, B], F32, tag="pawt", bufs=1, name=f"p_gh{k}")
        gate_matmuls(p_gh, c, False, True)
        nc.scalar.activation(hn_sb[:], p_gh[:], AF.Identity,
                             bias=vb[:, VB_HN + k:VB_HN + k + 1])
        nc.vector.tensor_mul(rhn_sb[:], rz_sb[:, k, :], hn_sb[:])
        p_gi = ppool.tile([128, B], F32, tag="pmain", bufs=2, name=f"p_gi{k}")
        gate_matmuls(p_gi, c, True, False)
        nc.vector.tensor_add(gin_sb[:], p_gi[:, 0:B], rhn_sb[:])
        nc.scalar.activation(n_sb[:, k, :], gin_sb[:], AF.Tanh,
                             bias=vb[:, VB_IN + k:VB_IN + k + 1])
        # new_h = n + z*(h - n)
        nc.vector.tensor_sub(rhn_sb[:], hT_sb[:, k, :], n_sb[:, k, :])
        nc.vector.tensor_mul(rhn_sb[:], rz_sb[:, 4 + k, :], rhn_sb[:])
        nc.vector.tensor_add(newhT_sb[:, k, :], n_sb[:, k, :], rhn_sb[:])

    # new_h natural + DMA
    for k in range(NK):
        p_tr = ppool.tile([128, 512], F32, tag="ptr", bufs=2, name=f"p_nh{k}")
        transpose_f32(p_tr[0:B, 0:128], newhT_sb[:, k, :], 128)
        nc.vector.tensor_copy(newh_nat[:, 128 * k:128 * (k + 1)], p_tr[0:B, 0:128])
    sync.dma_start(d_newh.ap(), newh_nat[:])

    # classifier + softmax
    p_lg = ppool.tile([128, B], F32, tag="pawt", bufs=1, name="p_lg")
    for k in range(NK):
        nc.tensor.matmul(p_lg[0:V, :], Wout_sb[:, k, :], newhT_sb[:, k, :],
                         start=(k == 0), stop=(k == NK - 1))
    nc.scalar.activation(lgT_sb[:], p_lg[0:V, :], AF.Identity,
                         bias=vb[0:V, VB_BO:VB_BO + 1])
    p_lgn = ppool.tile([128, 512], F32, tag="ptr", bufs=2, name="p_lgn")
    transpose_f32(p_lgn[0:B, 0:V], lgT_sb[:], V)
    nc.vector.tensor_copy(lg_nat[:], p_lgn[0:B, 0:V])

    nc.vector.tensor_reduce(smax[:], lg_nat[:], axis=AX.X, op=ALU.max)
    nc.vector.tensor_scalar_mul(snegmax[:], smax[:], -1.0)
    nc.scalar.activation(expv[:], lg_nat[:], AF.Exp, bias=snegmax[:],
                         accum_out=ssum[:])
    nc.vector.reciprocal(srecip[:], ssum[:])
    nc.vector.tensor_scalar(probs_sb[:], expv[:], srecip[:], None, op0=ALU.mult)
    sync.dma_start(d_probs.ap(), probs_sb[:])
    sync.dma_start(d_rinv.ap(), rinv_all[:])


_CACHED = None


def _get_program():
    global _CACHED
    if _CACHED is None:
        _CACHED = build_program()
    return _CACHED


def make_in_maps(inputs):
    inp = {k: np.ascontiguousarray(np.asarray(v, dtype=np.float32))
           for k, v in inputs.items()}
    vecbank = np.zeros((128, VB_N), np.float32)
    wv_bank = np.zeros((128, NK), np.float32)
    wv = inp["w_v"].reshape(H)
    ab = inp["b_hp"] + inp["b_ep"]
    brz = (inp["b_ih"] + inp["b_hh"])[0:2 * H]
    bin_ = inp["b_ih"][2 * H:]
    bhn = inp["b_hh"][2 * H:]
    for j in range(NK):
        wv_bank[:, j] = wv[128 * j:128 * (j + 1)]
        vecbank[:, VB_AB + j] = ab[128 * j:128 * (j + 1)]
        vecbank[:, VB_IN + j] = bin_[128 * j:128 * (j + 1)]
        vecbank[:, VB_HN + j] = bhn[128 * j:128 * (j + 1)]
    for c in range(8):
        vecbank[:, VB_RZ + c] = brz[128 * c:128 * (c + 1)]
    bcs = inp["b_cs"]
    vecbank[0:128, VB_CS + 0] = bcs[0:128]
    vecbank[0:CS - 128, VB_CS + 1] = bcs[128:CS]
    vecbank[0:V, VB_BO] = inp["b_out"]

    shared = {
        "W_hp": inp["W_hp"], "W_ep": inp["W_ep"], "W_cs": inp["W_cs"],
        "emb": inp["emb"], "W_ih": inp["W_ih"], "W_hh": inp["W_hh"],
        "W_out": inp["W_out"], "vecbank": vecbank, "wv_bank": wv_bank,
        "ident_r": np.eye(128, dtype=np.float32),
    }
    in_maps = []
    for c in range(N_CORES):
        sl = slice(B * c, B * (c + 1))
        m = dict(shared)
        m["in_char"] = np.ascontiguousarray(inp["in_char"][sl])
        m["hidden"] = np.ascontiguousarray(inp["hidden"][0, sl])
        m["enc"] = np.ascontiguousarray(inp["encoder_output"][:, sl])
        in_maps.append(m)
    return in_maps


def finish_outputs(core_results):
    probs = np.concatenate([r["out_probs"] for r in core_results], axis=0)
    newh = np.concatenate([r["out_newh"] for r in core_results], axis=0)[None]
    aw = []
    for r in core_results:
        expw = r["out_expw"]                      # [B, T], unnormalised
        rinv = r["out_rinv"]                      # [1, B]
        aw.append(expw * rinv.reshape(B, 1))
    attnw = np.concatenate(aw, axis=0)
    return probs, newh, attnw


def run(inputs, trace=False):
    nc = _get_program()
    in_maps = make_in_maps(inputs)
    res = bass_utils.run_bass_kernel_spmd(
        nc, in_maps, core_ids=list(range(N_CORES)), trace=trace)
    return finish_outputs(res.results), res


def kernel(**inputs):
    out, _ = run(inputs)
    return out


# revision 13
# speedup vs baseline: 190.5320x; 190.5320x over previous
"""Trainium2 Bass kernel for a single-step attention GRU decoder.

Math (per batch row b):
    A        = hidden @ W_hp + b_hp + b_ep                  [B, H]
    enc_attn = enc[t,b,:] @ W_ep                            [T, B, H]
    scores   = tanh(enc_attn + A[b]) @ w_v                  [B, T]
    attnw    = softmax(scores, axis=t)                      [B, T]   (output 3)
    context  = sum_t attnw[b,t] * enc[t,b,:]                [B, H]
    cs       = context @ W_cs + b_cs                        [B, 250]
    in_dec   = concat(emb[argmax(in_char)], cs)             [B, 300]
    GRU(in_dec, hidden) -> new_h                            [B, H]   (output 2)
    probs    = softmax(new_h @ W_out + b_out)               [B, 83]  (output 1)

Sharding: data-parallel over B=256 across 8 cores (32 rows each); all
weights replicated.  Inside a core everything runs in a "transposed"
layout (feature dim on partitions, batch on the free dim) so weight
matrices are stationary matmul operands in their natural [K_in, M_out]
layout and per-feature biases are per-partition scalars fused into
ScalarE activations.

enc (39 MB/core) is DMAed in natural layout [t, h] per batch row,
transposed on the PE (128x128 identity matmuls) to get enc^T [h, t] for
the score matmul; the natural copy is reused as the moving operand of
the context matmul (stationary = attnw^T column, contraction over t).
The big score matmul runs in float32r (single-pass fp32, 1 cycle/row at
N>=256 vs 4 for full fp32).

Engine APs can only start at partitions {0,32,64,96}, so per-row work
is placed at those bases (4 batch rows per "quad"), single rows move
between partitions via SBUF<->SBUF DMA, and the attention softmax is
only computed up to exp() on device - the 1/sum normalisation of the
attnw output happens on the host while the context matmul folds the
normalisation into its PSUM->SBUF eviction (ScalarE scale).
"""

import numpy as np

import concourse.bass as bass
import concourse.bacc as bacc
import concourse.mybir as mybir
import concourse.tile as tile
from concourse import bass_utils, masks

dt = mybir.dt
F32 = dt.float32
F32R = dt.float32r
AF = mybir.ActivationFunctionType
ALU = mybir.AluOpType
AX = mybir.AxisListType

N_CORES = 8
B = 32          # per-core batch
T = 600
H = 512
V = 83
EMB = 50
CS = 250        # 5*EMB
GI = 300        # 6*EMB
G3 = 3 * H      # 1536

TCH = [(0, 128), (128, 128), (256, 128), (384, 128), (512, 88)]
NK = H // 128   # 4 h chunks

# vecbank columns
VB_WV = 0       # 4: w_v chunks
VB_AB = 4       # 4: b_hp + b_ep chunks
VB_RZ = 8       # 8: (b_ih + b_hh)[0:1024] chunks
VB_IN = 16      # 4: b_ih[1024:1536] chunks
VB_HN = 20      # 4: b_hh[1024:1536] chunks
VB_CS = 24      # 2: b_cs chunks (128, 122)
VB_BO = 26      # 1: b_out (83)
VB_N = 27

# in_dec^T partition chunks: [emb(50) | cs 0:128 | cs 128:250]
GIK = [EMB, 128, CS - 128]          # 50, 128, 122
CS_SPLIT = [(0, 128, 1), (128, CS - 128, 2)]   # (cs_off, n_rows, chunk_idx)


def _r(ap):
    return ap.bitcast(F32R)


def build_program(loop_n=1):
    nc = bacc.Bacc("TRN2", target_bir_lowering=False, debug=False,
                   num_devices=N_CORES)

    din = {}
    def inp(name, shape):
        din[name] = nc.dram_tensor(name, list(shape), F32, kind="ExternalInput")
        return din[name]

    inp("in_char", (B, V))
    inp("hidden", (B, H))
    din["enc"] = nc.dram_tensor("enc", [T, B, H], F32R, kind="ExternalInput")
    inp("W_hp", (H, H))
    din["W_ep"] = nc.dram_tensor("W_ep", [H, H], F32R, kind="ExternalInput")
    din["wv_bank"] = nc.dram_tensor("wv_bank", [128, NK], F32R, kind="ExternalInput")
    din["ident_r"] = nc.dram_tensor("ident_r", [128, 128], F32R, kind="ExternalInput")
    inp("W_cs", (H, CS))
    inp("emb", (V, EMB))
    inp("W_ih", (GI, G3))        # device re-chunks rows to (50,128,122)
    inp("W_hh", (H, G3))
    inp("W_out", (H, V))
    inp("vecbank", (128, VB_N))

    d_probs = nc.dram_tensor("out_probs", [B, V], F32, kind="ExternalOutput")
    d_newh = nc.dram_tensor("out_newh", [B, H], F32, kind="ExternalOutput")
    d_expw = nc.dram_tensor("out_expw", [B, T], F32, kind="ExternalOutput")
    d_rinv = nc.dram_tensor("out_rinv", [1, B], F32, kind="ExternalOutput")

    with tile.TileContext(nc) as tc:
        with (
            tc.tile_pool(name="const", bufs=1) as cpool,
            tc.tile_pool(name="enc", bufs=6) as encpool,
            tc.tile_pool(name="work", bufs=2) as wpool,
            tc.tile_pool(name="psum", bufs=1, space=bass.MemorySpace.PSUM) as ppool,
        ):
            build_body(nc, tc, cpool, encpool, wpool, ppool, din,
                       d_probs, d_newh, d_expw, d_rinv, loop_n=loop_n)

    nc.compile()
    return nc


def build_body(nc, tc, cpool, encpool, wpool, ppool, din,
               d_probs, d_newh, d_expw, d_rinv, loop_n=1):
    sync = nc.sync

    # ---------------- constants into SBUF ----------------
    Wep_sb = cpool.tile([128, NK, H], F32R)
    sync.dma_start(Wep_sb[:], din["W_ep"].ap().rearrange("(k p) n -> p k n", p=128))
    Whp_sb = cpool.tile([128, NK, H], F32)
    sync.dma_start(Whp_sb[:], din["W_hp"].ap().rearrange("(k p) n -> p k n", p=128))
    Wcs_sb = cpool.tile([128, NK, CS], F32)
    sync.dma_start(Wcs_sb[:], din["W_cs"].ap().rearrange("(k p) n -> p k n", p=128))
    Whh_sb = cpool.tile([128, NK, G3], F32)
    sync.dma_start(Whh_sb[:], din["W_hh"].ap().rearrange("(k p) n -> p k n", p=128))
    Wout_sb = cpool.tile([128, NK, V], F32)
    sync.dma_start(Wout_sb[:], din["W_out"].ap().rearrange("(k p) n -> p k n", p=128))
    Wih_sb = cpool.tile([128, 3, G3], F32)
    sync.dma_start(Wih_sb[0:GIK[0], 0, :], din["W_ih"][0:50, :])
    sync.dma_start(Wih_sb[0:GIK[1], 1, :], din["W_ih"][50:178, :])
    sync.dma_start(Wih_sb[0:GIK[2], 2, :], din["W_ih"][178:300, :])
    emb_sb = cpool.tile([V, EMB], F32)
    sync.dma_start(emb_sb[:], din["emb"].ap())
    vb = cpool.tile([128, VB_N], F32)
    sync.dma_start(vb[:], din["vecbank"].ap())
    wv_sb = cpool.tile([128, NK], F32R)
    sync.dma_start(wv_sb[:], din["wv_bank"].ap())
    ic_sb = cpool.tile([B, V], F32)
    sync.dma_start(ic_sb[:], din["in_char"].ap())
    hid_sb = cpool.tile([B, H], F32)
    sync.dma_start(hid_sb[:], din["hidden"].ap())

    ident = cpool.tile([128, 128], F32)
    masks.make_identity(nc, ident[:])
    ident_r = cpool.tile([128, 128], F32R)
    sync.dma_start(ident_r[:], din["ident_r"].ap())
    ones_sb = cpool.tile([128, 1], F32)
    nc.gpsimd.memset(ones_sb[:], 1.0)

    # persistent per-core intermediates
    AT_sb = cpool.tile([128, NK, B], F32)        # (hid @ W_hp + b_hp + b_ep)^T
    hT_sb = cpool.tile([128, NK, B], F32)
    rinv_all = cpool.tile([1, B], F32)           # 1/sum(exp(scores)) per row
    ctx_nat = cpool.tile([B, H], F32)            # normalised context rows
    ctxT_sb = cpool.tile([128, NK, B], F32)
    indec_sb = cpool.tile([128, 3, B], F32)
    rz_sb = cpool.tile([128, 8, B], F32)         # r gates 0:4, z gates 4:8
    n_sb = cpool.tile([128, NK, B], F32)
    newhT_sb = cpool.tile([128, NK, B], F32)
    newh_nat = cpool.tile([B, H], F32)
    lgT_sb = cpool.tile([V, B], F32)
    lg_nat = cpool.tile([B, V], F32)
    expv = cpool.tile([B, V], F32)
    probs_sb = cpool.tile([B, V], F32)
    smax = cpool.tile([B, 1], F32)
    snegmax = cpool.tile([B, 1], F32)
    ssum = cpool.tile([B, 1], F32)
    srecip = cpool.tile([B, 1], F32)

    def transpose_f32(psum_out, in_ap, n_rows):
        # psum_out <- in_ap.T ; in_ap is [n_rows, M] at partition base 0
        nc.tensor.transpose(psum_out, in_ap, ident[0:n_rows, 0:n_rows])

    def transpose_f32r(psum_out, in_ap, n_rows):
        nc.tensor.transpose(psum_out, in_ap, ident_r[0:n_rows, 0:n_rows])

    import contextlib
    loop_ctx = (tc.For_i(0, loop_n, 1) if loop_n > 1
                else contextlib.nullcontext())
    with loop_ctx:
        body_main(nc, tc, cpool, encpool, wpool, ppool, din,
                  d_probs, d_newh, d_expw, d_rinv,
                  transpose_f32, transpose_f32r,
                  Wep_sb, Whp_sb, Wcs_sb, Whh_sb, Wout_sb, Wih_sb, emb_sb,
                  vb, wv_sb, ic_sb, hid_sb, ones_sb,
                  AT_sb, hT_sb, rinv_all, ctx_nat, ctxT_sb, indec_sb, rz_sb,
                  n_sb, newhT_sb, newh_nat, lgT_sb, lg_nat, expv, probs_sb,
                  smax, snegmax, ssum, srecip)


def body_main(nc, tc, cpool, encpool, wpool, ppool, din,
              d_probs, d_newh, d_expw, d_rinv,
              transpose_f32, transpose_f32r,
              Wep_sb, Whp_sb, Wcs_sb, Whh_sb, Wout_sb, Wih_sb, emb_sb,
              vb, wv_sb, ic_sb, hid_sb, ones_sb,
              AT_sb, hT_sb, rinv_all, ctx_nat, ctxT_sb, indec_sb, rz_sb,
              n_sb, newhT_sb, newh_nat, lgT_sb, lg_nat, expv, probs_sb,
              smax, snegmax, ssum, srecip):
    sync = nc.sync
    # ---------------- stage A: hT, A^T, embedding ----------------
    for k in range(NK):
        p_tr = ppool.tile([128, 512], F32, tag="ptr", bufs=2, name=f"p_hT{k}")
        transpose_f32(p_tr[:, 0:B], hid_sb[:, 128 * k:128 * (k + 1)], B)
        nc.vector.tensor_copy(hT_sb[:, k, :], p_tr[:, 0:B])

    for j in range(NK):
        p_a = ppool.tile([128, B], F32, tag="pawt", bufs=1, name=f"p_A{j}")
        for k in range(NK):
            nc.tensor.matmul(p_a[:], Whp_sb[:, k, 128 * j:128 * (j + 1)],
                             hT_sb[:, k, :], start=(k == 0), stop=(k == NK - 1))
        nc.scalar.activation(AT_sb[:, j, :], p_a[:], AF.Identity,
                             bias=vb[:, VB_AB + j:VB_AB + j + 1])

    # argmax(in_char) -> one-hot -> emb rows into in_dec^T chunk 0
    icmax = cpool.tile([B, 1], F32)
    onehot = cpool.tile([B, V], F32)
    onehotT = cpool.tile([V, B], F32)
    nc.vector.tensor_reduce(icmax[:], ic_sb[:], axis=AX.X, op=ALU.max)
    nc.vector.tensor_scalar(onehot[:], ic_sb[:], icmax[:], None, op0=ALU.is_ge)
    p_oh = ppool.tile([128, B], F32, tag="pawt", bufs=1, name="p_oh")
    transpose_f32(p_oh[0:V, :], onehot[:], B)
    nc.vector.tensor_copy(onehotT[:], p_oh[0:V, :])
    p_et = ppool.tile([128, B], F32, tag="pawt", bufs=1, name="p_et")
    nc.tensor.matmul(p_et[0:EMB, :], emb_sb[:], onehotT[:], start=True, stop=True)
    nc.scalar.copy(indec_sb[0:EMB, 0, :], p_et[0:EMB, :])

    # ---------------- stage B: per-quad attention pipeline ----------------
    for q in range(B // 4):
        enc_nats = []
        expw_p0s = []
        for bi in range(4):
            b = 4 * q + bi
            # natural enc_b: [128, 5, 512]; row p of chunk j is t = 128j + p
            enc_nat = encpool.tile([128, len(TCH), H], F32R, tag="enc_nat",
                                   name=f"enc_nat{b}")
            enc_nats.append(enc_nat)
            sync.dma_start(
                enc_nat[:, 0:4, :],
                din["enc"][0:512, b, :].rearrange("(k p) c -> p k c", p=128))
            sync.dma_start(enc_nat[0:88, 4, :], din["enc"][512:600, b, :])

            # enc^T via PE transposes
            encT = wpool.tile([128, NK, T], F32R, tag="encT", name=f"encT{b}")
            for k in range(NK):
                p_tr = ppool.tile([128, 512], F32R, tag="ptr", bufs=2,
                                  name=f"p_tr{b}_{k}")
                for (ti, (t0, tw)) in enumerate(TCH[:4]):
                    transpose_f32r(p_tr[:, 128 * ti:128 * ti + tw],
                                   enc_nat[0:tw, ti, 128 * k:128 * (k + 1)], tw)
                p_tr2 = ppool.tile([128, 512], F32R, tag="ptr", bufs=2,
                                   name=f"p_tr2{b}_{k}")
                transpose_f32r(p_tr2[:, 0:88],
                               enc_nat[0:88, 4, 128 * k:128 * (k + 1)], 88)
                if k % 2 == 0:
                    nc.vector.tensor_copy(encT[:, k, 0:512], p_tr[:])
                    nc.vector.tensor_copy(encT[:, k, 512:600], p_tr2[:, 0:88])
                else:
                    nc.scalar.copy(encT[:, k, 0:512], p_tr[:])
                    nc.scalar.copy(encT[:, k, 512:600], p_tr2[:, 0:88])

            # enc_attn^T[j] = W_ep[:,j].T @ enc^T ; then tanh(+A bias)
            tanhT = wpool.tile([128, NK, T], F32R, tag="tanhT", name=f"tanhT{b}")
            for j in range(NK):
                for (t0, tw) in ((0, 300), (300, 300)):
                    p_m = ppool.tile([128, 300], F32, tag="pmain", bufs=2,
                                     name=f"p_m{b}_{j}_{t0}")
                    for k in range(NK):
                        nc.tensor.matmul(
                            p_m[:],
                            Wep_sb[:, k, 128 * j:128 * (j + 1)],
                            encT[:, k, t0:t0 + tw],
                            start=(k == 0), stop=(k == NK - 1))
                    nc.scalar.activation(tanhT[:, j, t0:t0 + tw], p_m[:], AF.Tanh,
                                         bias=AT_sb[:, j, b:b + 1])

            # scores row (partition 0): w_v^T @ tanh^T
            qscA = ppool.tile([1, 512], F32, tag="qscA", bufs=1, name=f"qscA{b}")
            qscB = ppool.tile([1, 512], F32, tag="qscB", bufs=1, name=f"qscB{b}")
            for (qsc, t0) in ((qscA, 0), (qscB, 300)):
                for j in range(NK):
                    nc.tensor.matmul(
                        qsc[0:1, 0:300],
                        wv_sb[:, j:j + 1],
                        tanhT[:, j, t0:t0 + 300],
                        start=(j == 0), stop=(j == NK - 1))

            # softmax pieces at partition 0
            # sred cols: 0,1 = halves' max, 2 = row max, 3 = -max
            sred = wpool.tile([1, 4], F32, tag="sred", name=f"sred{b}")
            nc.vector.tensor_reduce(sred[0:1, 0:1], qscA[0:1, 0:300],
                                    axis=AX.X, op=ALU.max)
            nc.vector.tensor_reduce(sred[0:1, 1:2], qscB[0:1, 0:300],
                                    axis=AX.X, op=ALU.max)
            nc.vector.tensor_reduce(sred[0:1, 2:3], sred[0:1, 0:2],
                                    axis=AX.X, op=ALU.max)
            nc.vector.tensor_scalar_mul(sred[0:1, 3:4], sred[0:1, 2:3], -1.0)
            expw_p0 = wpool.tile([1, T], F32, tag="expw_p0", bufs=3,
                                 name=f"expw_p0{b}")
            expw_p0s.append(expw_p0)
            ssum2 = wpool.tile([1, 2], F32, tag="ssum2", name=f"ssum2{b}")
            nc.scalar.activation(expw_p0[0:1, 0:300], qscA[0:1, 0:300], AF.Exp,
                                 bias=sred[0:1, 3:4], accum_out=ssum2[0:1, 0:1])
            nc.scalar.activation(expw_p0[0:1, 300:600], qscB[0:1, 0:300], AF.Exp,
                                 bias=sred[0:1, 3:4], accum_out=ssum2[0:1, 1:2])
            stot = wpool.tile([1, 1], F32, tag="stot", name=f"stot{b}")
            nc.vector.tensor_reduce(stot[0:1, :], ssum2[0:1, 0:2],
                                    axis=AX.X, op=ALU.add)
            nc.vector.reciprocal(rinv_all[0:1, b:b + 1], stot[0:1, :])
            # unnormalised attention row -> DRAM (host divides by the sum)
            sync.dma_start(d_expw[b:b + 1, :], expw_p0[0:1, :])

        # attnw^T columns via ones outer-product (partition 0 only)
        awT = wpool.tile([128, len(TCH), 4], F32R, tag="awT", name=f"awT{q}")
        p_awt = ppool.tile([128, 32], F32, tag="pawt", bufs=1, name=f"p_awt{q}")
        for bi in range(4):
            for (ti, (t0, tw)) in enumerate(TCH):
                nc.tensor.matmul(p_awt[0:tw, 4 * ti + bi:4 * ti + bi + 1],
                                 expw_p0s[bi][0:1, t0:t0 + tw],
                                 ones_sb[0:1, 0:1],
                                 start=True, stop=True)
        for (ti, (t0, tw)) in enumerate(TCH):
            nc.vector.tensor_copy(awT[0:tw, ti, :],
                                  p_awt[0:tw, 4 * ti:4 * ti + 4])

        # context rows: ctx[b] = rinv * sum_t expw[b,t] * enc[t,b,:]
        for bi in range(4):
            b = 4 * q + bi
            pctx = ppool.tile([1, 512], F32, tag="pctx", bufs=1, name=f"pctx{b}")
            for (ti, (t0, tw)) in enumerate(TCH):
                nc.tensor.matmul(pctx[0:1, :],
                                 awT[0:tw, ti, bi:bi + 1],
                                 enc_nats[bi][0:tw, ti, :],
                                 start=(ti == 0), stop=(ti == len(TCH) - 1))
            ctx_p0 = wpool.tile([1, H], F32, tag="ctx_p0", bufs=2,
                                name=f"ctx_p0{b}")
            nc.scalar.activation(ctx_p0[0:1, :], pctx[0:1, :],
                                 AF.Copy, scale=rinv_all[0:1, b:b + 1])
            # move the row into place (DMA crosses partitions freely)
            sync.dma_start(ctx_nat[b:b + 1, :], ctx_p0[0:1, :])

    # ---------------- stage C: epilogue ----------------
    for k in range(NK):
        p_tr = ppool.tile([128, 512], F32, tag="ptr", bufs=2, name=f"p_ctxT{k}")
        transpose_f32(p_tr[:, 0:B], ctx_nat[:, 128 * k:128 * (k + 1)], B)
        nc.vector.tensor_copy(ctxT_sb[:, k, :], p_tr[:, 0:B])

    # context shrink into in_dec^T chunks 1,2
    for (cs0, n_rows, ci) in CS_SPLIT:
        p_cs = ppool.tile([128, B], F32, tag="pawt", bufs=1, name=f"p_cs{ci}")
        for k in range(NK):
            nc.tensor.matmul(p_cs[0:n_rows, :],
                             Wcs_sb[:, k, cs0:cs0 + n_rows],
                             ctxT_sb[:, k, :],
                             start=(k == 0), stop=(k == NK - 1))
        nc.scalar.activation(indec_sb[0:n_rows, ci, :],
                             p_cs[0:n_rows, :], AF.Identity,
                             bias=vb[0:n_rows, VB_CS + ci - 1:VB_CS + ci])

    # GRU gates, chunks of 128 over 3H
    def gate_matmuls(p_g, c, with_ih, with_hh):
        nmm = (3 if with_ih else 0) + (NK if with_hh else 0)
        i = 0
        if with_ih:
            for k in range(3):
                kw = GIK[k]
                nc.tensor.matmul(p_g[:], Wih_sb[0:kw, k, 128 * c:128 * (c + 1)],
                                 indec_sb[0:kw, k, :],
                                 start=(i == 0), stop=(i == nmm - 1))
                i += 1
        if with_hh:
            for k in range(NK):
                nc.tensor.matmul(p_g[:], Whh_sb[:, k, 128 * c:128 * (c + 1)],
                                 hT_sb[:, k, :],
                                 start=(i == 0), stop=(i == nmm - 1))
                i += 1

    for c in range(8):  # r and z gates
        p_g = ppool.tile([128, B], F32, tag="pawt", bufs=1, name=f"p_g{c}")
        gate_matmuls(p_g, c, True, True)
        nc.scalar.activation(rz_sb[:, c, :], p_g[:], AF.Sigmoid,
                             bias=vb[:, VB_RZ + c:VB_RZ + c + 1])

    hn_sb = cpool.tile([128, B], F32)
    rhn_sb = cpool.tile([128, B], F32)
    gin_sb = cpool.tile([128, B], F32)
    for k in range(NK):  # n gate chunks + new_h
        c = 8 + k
        p_gh = ppool.tile([128, B], F32, tag="pawt", bufs=1, name=f"p_gh{k}")
        gate_matmuls(p_gh, c, False, True)
        nc.scalar.activation(hn_sb[:], p_gh[:], AF.Identity,
                             bias=vb[:, VB_HN + k:VB_HN + k + 1])
        nc.vector.tensor_mul(rhn_sb[:], rz_sb[:, k, :], hn_sb[:])
        p_gi = ppool.tile([128, B], F32, tag="pmain", bufs=2, name=f"p_gi{k}")
        gate_matmuls(p_gi, c, True, False)
        nc.vector.tensor_add(gin_sb[:], p_gi[:, 0:B], rhn_sb[:])
        nc.scalar.activation(n_sb[:, k, :], gin_sb[:], AF.Tanh,
                             bias=vb[:, VB_IN + k:VB_IN + k + 1])
        # new_h = n + z*(h - n)
        nc.vector.tensor_sub(rhn_sb[:], hT_sb[:, k, :], n_sb[:, k, :])
        nc.vector.tensor_mul(rhn_sb[:], rz_sb[:, 4 + k, :], rhn_sb[:])
        nc.vector.tensor_add(newhT_sb[:, k, :], n_sb[:, k, :], rhn_sb[:])

    # new_h natural + DMA
    for k in range(NK):
        p_tr = ppool.tile([128, 512], F32, tag="ptr", bufs=2, name=f"p_nh{k}")
        transpose_f32(p_tr[0:B, 0:128], newhT_sb[:, k, :], 128)
        nc.vector.tensor_copy(newh_nat[:, 128 * k:128 * (k + 1)], p_tr[0:B, 0:128])
    sync.dma_start(d_newh.ap(), newh_nat[:])

    # classifier + softmax
    p_lg = ppool.tile([128, B], F32, tag="pawt", bufs=1, name="p_lg")
    for k in range(NK):
        nc.tensor.matmul(p_lg[0:V, :], Wout_sb[:, k, :], newhT_sb[:, k, :],
                         start=(k == 0), stop=(k == NK - 1))
    nc.scalar.activation(lgT_sb[:], p_lg[0:V, :], AF.Identity,
                         bias=vb[0:V, VB_BO:VB_BO + 1])
    p_lgn = ppool.tile([128, 512], F32, tag="ptr", bufs=2, name="p_lgn")
    transpose_f32(p_lgn[0:B, 0:V], lgT_sb[:], V)
    nc.vector.tensor_copy(lg_nat[:], p_lgn[0:B, 0:V])

    nc.vector.tensor_reduce(smax[:], lg_nat[:], axis=AX.X, op=ALU.max)
    nc.vector.tensor_scalar_mul(snegmax[:], smax[:], -1.0)
    nc.scalar.activation(expv[:], lg_nat[:], AF.Exp, bias=snegmax[:],
                         accum_out=ssum[:])
    nc.vector.reciprocal(srecip[:], ssum[:])
    nc.vector.tensor_scalar(probs_sb[:], expv[:], srecip[:], None, op0=ALU.mult)
    sync.dma_start(d_probs.ap(), probs_sb[:])
    sync.dma_start(d_rinv.ap(), rinv_all[:])


_CACHED = None


def _get_program():
    global _CACHED
    if _CACHED is None:
        _CACHED = build_program()
    return _CACHED


def make_in_maps(inputs):
    inp = {k: np.ascontiguousarray(np.asarray(v, dtype=np.float32))
           for k, v in inputs.items()}
    vecbank = np.zeros((128, VB_N), np.float32)
    wv_bank = np.zeros((128, NK), np.float32)
    wv = inp["w_v"].reshape(H)
    ab = inp["b_hp"] + inp["b_ep"]
    brz = (inp["b_ih"] + inp["b_hh"])[0:2 * H]
    bin_ = inp["b_ih"][2 * H:]
    bhn = inp["b_hh"][2 * H:]
    for j in range(NK):
        wv_bank[:, j] = wv[128 * j:128 * (j + 1)]
        vecbank[:, VB_AB + j] = ab[128 * j:128 * (j + 1)]
        vecbank[:, VB_IN + j] = bin_[128 * j:128 * (j + 1)]
        vecbank[:, VB_HN + j] = bhn[128 * j:128 * (j + 1)]
    for c in range(8):
        vecbank[:, VB_RZ + c] = brz[128 * c:128 * (c + 1)]
    bcs = inp["b_cs"]
    vecbank[0:128, VB_CS + 0] = bcs[0:128]
    vecbank[0:CS - 128, VB_CS + 1] = bcs[128:CS]
    vecbank[0:V, VB_BO] = inp["b_out"]

    shared = {
        "W_hp": inp["W_hp"], "W_ep": inp["W_ep"], "W_cs": inp["W_cs"],
        "emb": inp["emb"], "W_ih": inp["W_ih"], "W_hh": inp["W_hh"],
        "W_out": inp["W_out"], "vecbank": vecbank, "wv_bank": wv_bank,
        "ident_r": np.eye(128, dtype=np.float32),
    }
    in_maps = []
    for c in range(N_CORES):
        sl = slice(B * c, B * (c + 1))
        m = dict(shared)
        m["in_char"] = np.ascontiguousarray(inp["in_char"][sl])
        m["hidden"] = np.ascontiguousarray(inp["hidden"][0, sl])
        m["enc"] = np.ascontiguousarray(inp["encoder_output"][:, sl])
        in_maps.append(m)
    return in_maps


def finish_outputs(core_results):
    probs = np.concatenate([r["out_probs"] for r in core_results], axis=0)
    newh = np.concatenate([r["out_newh"] for r in core_results], axis=0)[None]
    aw = []
    for r in core_results:
        expw = r["out_expw"]                      # [B, T], unnormalised
        rinv = r["out_rinv"]                      # [1, B]
        aw.append(expw * rinv.reshape(B, 1))
    attnw = np.concatenate(aw, axis=0)
    return probs, newh, attnw


def run(inputs, trace=False):
    nc = _get_program()
    in_maps = make_in_maps(inputs)
    res = bass_utils.run_bass_kernel_spmd(
        nc, in_maps, core_ids=list(range(N_CORES)), trace=trace)
    return finish_outputs(res.results), res


def kernel(**inputs):
    out, _ = run(inputs)
    return out


# revision 15
# speedup vs baseline: 246.4508x; 1.2935x over previous
"""Trainium2 Bass kernel for a single-step attention GRU decoder.

Math (per batch row b):
    A        = hidden @ W_hp + b_hp + b_ep                  [B, H]
    enc_attn = enc[t,b,:] @ W_ep                            [T, B, H]
    scores   = tanh(enc_attn + A[b]) @ w_v                  [B, T]
    attnw    = softmax(scores, axis=t)                      [B, T]   (output 3)
    context  = sum_t attnw[b,t] * enc[t,b,:]                [B, H]
    cs       = context @ W_cs + b_cs                        [B, 250]
    in_dec   = concat(emb[argmax(in_char)], cs)             [B, 300]
    GRU(in_dec, hidden) -> new_h                            [B, H]   (output 2)
    probs    = softmax(new_h @ W_out + b_out)               [B, 83]  (output 1)

Sharding: data-parallel over B=256 across 8 cores (32 rows each); all
weights replicated.  Inside a core everything runs in a "transposed"
layout (feature dim on partitions, batch on the free dim) so weight
matrices are stationary matmul operands in their natural [K_in, M_out]
layout and per-feature biases are per-partition scalars fused into
ScalarE activations.

enc (39 MB/core) is DMAed in natural layout [t, h] per batch row,
transposed on the PE (128x128 identity matmuls) to get enc^T [h, t] for
the score matmul; the natural copy is reused as the moving operand of
the context matmul (stationary = attnw^T column, contraction over t).
The big score matmul runs in float32r (single-pass fp32, 1 cycle/row at
N>=256 vs 4 for full fp32).

Engine APs can only start at partitions {0,32,64,96}, so per-row work
is placed at those bases (4 batch rows per "quad"), single rows move
between partitions via SBUF<->SBUF DMA, and the attention softmax is
only computed up to exp() on device - the 1/sum normalisation of the
attnw output happens on the host while the context matmul folds the
normalisation into its PSUM->SBUF eviction (ScalarE scale).
"""

import ml_dtypes
import numpy as np

import concourse.bass as bass
import concourse.bacc as bacc
import concourse.mybir as mybir
import concourse.tile as tile
from concourse import bass_utils, masks

dt = mybir.dt
F32 = dt.float32
F32R = dt.float32r
BF16 = dt.bfloat16
AF = mybir.ActivationFunctionType
ALU = mybir.AluOpType
AX = mybir.AxisListType

N_CORES = 8
B = 32          # per-core batch
T = 600
H = 512
V = 83
EMB = 50
CS = 250        # 5*EMB
GI = 300        # 6*EMB
G3 = 3 * H      # 1536

TCH = [(0, 128), (128, 128), (256, 128), (384, 128), (512, 88)]
NK = H // 128   # 4 h chunks

# vecbank columns
VB_WV = 0       # 4: w_v chunks
VB_AB = 4       # 4: b_hp + b_ep chunks
VB_RZ = 8       # 8: (b_ih + b_hh)[0:1024] chunks
VB_IN = 16      # 4: b_ih[1024:1536] chunks
VB_HN = 20      # 4: b_hh[1024:1536] chunks
VB_CS = 24      # 2: b_cs chunks (128, 122)
VB_BO = 26      # 1: b_out (83)
VB_N = 27

# in_dec^T partition chunks: [emb(50) | cs 0:128 | cs 128:250]
GIK = [EMB, 128, CS - 128]          # 50, 128, 122
CS_SPLIT = [(0, 128, 1), (128, CS - 128, 2)]   # (cs_off, n_rows, chunk_idx)


def _r(ap):
    return ap.bitcast(F32R)


def build_program(loop_n=1):
    nc = bacc.Bacc("TRN2", target_bir_lowering=False, debug=False,
                   num_devices=N_CORES)

    din = {}
    def inp(name, shape):
        din[name] = nc.dram_tensor(name, list(shape), F32, kind="ExternalInput")
        return din[name]

    inp("in_char", (B, V))
    inp("hidden", (B, H))
    inp("enc", (T, B, H))
    inp("W_hp", (H, H))
    din["W_ep"] = nc.dram_tensor("W_ep", [H, H], BF16, kind="ExternalInput")
    din["wv_bank"] = nc.dram_tensor("wv_bank", [128, NK], BF16, kind="ExternalInput")
    inp("W_cs", (H, CS))
    inp("emb", (V, EMB))
    inp("W_ih", (GI, G3))        # device re-chunks rows to (50,128,122)
    inp("W_hh", (H, G3))
    inp("W_out", (H, V))
    inp("vecbank", (128, VB_N))

    d_probs = nc.dram_tensor("out_probs", [B, V], F32, kind="ExternalOutput")
    d_newh = nc.dram_tensor("out_newh", [B, H], F32, kind="ExternalOutput")
    d_expw = nc.dram_tensor("out_expw", [B, T], F32, kind="ExternalOutput")
    d_rinv = nc.dram_tensor("out_rinv", [1, B], F32, kind="ExternalOutput")

    with tile.TileContext(nc) as tc:
        with (
            tc.tile_pool(name="const", bufs=1) as cpool,
            tc.tile_pool(name="enc", bufs=6) as encpool,
            tc.tile_pool(name="work", bufs=2) as wpool,
            tc.tile_pool(name="psum", bufs=1, space=bass.MemorySpace.PSUM) as ppool,
        ):
            build_body(nc, tc, cpool, encpool, wpool, ppool, din,
                       d_probs, d_newh, d_expw, d_rinv, loop_n=loop_n)

    nc.compile()
    return nc


def build_body(nc, tc, cpool, encpool, wpool, ppool, din,
               d_probs, d_newh, d_expw, d_rinv, loop_n=1):
    sync = nc.sync

    # ---------------- constants into SBUF ----------------
    Wep_sb = cpool.tile([128, NK, H], BF16)
    sync.dma_start(Wep_sb[:], din["W_ep"].ap().rearrange("(k p) n -> p k n", p=128))
    Whp_sb = cpool.tile([128, NK, H], F32)
    sync.dma_start(Whp_sb[:], din["W_hp"].ap().rearrange("(k p) n -> p k n", p=128))
    Wcs_sb = cpool.tile([128, NK, CS], F32)
    sync.dma_start(Wcs_sb[:], din["W_cs"].ap().rearrange("(k p) n -> p k n", p=128))
    Whh_sb = cpool.tile([128, NK, G3], F32)
    sync.dma_start(Whh_sb[:], din["W_hh"].ap().rearrange("(k p) n -> p k n", p=128))
    Wout_sb = cpool.tile([128, NK, V], F32)
    sync.dma_start(Wout_sb[:], din["W_out"].ap().rearrange("(k p) n -> p k n", p=128))
    Wih_sb = cpool.tile([128, 3, G3], F32)
    sync.dma_start(Wih_sb[0:GIK[0], 0, :], din["W_ih"][0:50, :])
    sync.dma_start(Wih_sb[0:GIK[1], 1, :], din["W_ih"][50:178, :])
    sync.dma_start(Wih_sb[0:GIK[2], 2, :], din["W_ih"][178:300, :])
    emb_sb = cpool.tile([V, EMB], F32)
    sync.dma_start(emb_sb[:], din["emb"].ap())
    vb = cpool.tile([128, VB_N], F32)
    sync.dma_start(vb[:], din["vecbank"].ap())
    wv_sb = cpool.tile([128, NK], BF16)
    sync.dma_start(wv_sb[:], din["wv_bank"].ap())
    ic_sb = cpool.tile([B, V], F32)
    sync.dma_start(ic_sb[:], din["in_char"].ap())
    hid_sb = cpool.tile([B, H], F32)
    sync.dma_start(hid_sb[:], din["hidden"].ap())

    ident = cpool.tile([128, 128], F32)
    masks.make_identity(nc, ident[:])
    ident_bf = cpool.tile([128, 128], BF16)
    masks.make_identity(nc, ident_bf[:])
    ones_sb = cpool.tile([128, 1], F32)
    nc.gpsimd.memset(ones_sb[:], 1.0)

    # persistent per-core intermediates
    AT_sb = cpool.tile([128, NK, B], F32)        # (hid @ W_hp + b_hp + b_ep)^T
    hT_sb = cpool.tile([128, NK, B], F32)
    rinv_all = cpool.tile([1, B], F32)           # 1/sum(exp(scores)) per row
    ctx_nat = cpool.tile([B, H], F32)            # normalised context rows
    ctxT_sb = cpool.tile([128, NK, B], F32)
    indec_sb = cpool.tile([128, 3, B], F32)
    rz_sb = cpool.tile([128, 8, B], F32)         # r gates 0:4, z gates 4:8
    n_sb = cpool.tile([128, NK, B], F32)
    newhT_sb = cpool.tile([128, NK, B], F32)
    newh_nat = cpool.tile([B, H], F32)
    lgT_sb = cpool.tile([V, B], F32)
    lg_nat = cpool.tile([B, V], F32)
    expv = cpool.tile([B, V], F32)
    probs_sb = cpool.tile([B, V], F32)
    smax = cpool.tile([B, 1], F32)
    snegmax = cpool.tile([B, 1], F32)
    ssum = cpool.tile([B, 1], F32)
    srecip = cpool.tile([B, 1], F32)

    def transpose_f32(psum_out, in_ap, n_rows):
        # psum_out <- in_ap.T ; in_ap is [n_rows, M] at partition base 0
        nc.tensor.transpose(psum_out, in_ap, ident[0:n_rows, 0:n_rows])

    def transpose_bf16(psum_out, in_ap, n_rows):
        nc.tensor.transpose(psum_out, in_ap, ident_bf[0:n_rows, 0:n_rows])

    import contextlib
    loop_ctx = (tc.For_i(0, loop_n, 1) if loop_n > 1
                else contextlib.nullcontext())
    with loop_ctx:
        body_main(nc, tc, cpool, encpool, wpool, ppool, din,
                  d_probs, d_newh, d_expw, d_rinv,
                  transpose_f32, transpose_bf16,
                  Wep_sb, Whp_sb, Wcs_sb, Whh_sb, Wout_sb, Wih_sb, emb_sb,
                  vb, wv_sb, ic_sb, hid_sb, ones_sb,
                  AT_sb, hT_sb, rinv_all, ctx_nat, ctxT_sb, indec_sb, rz_sb,
                  n_sb, newhT_sb, newh_nat, lgT_sb, lg_nat, expv, probs_sb,
                  smax, snegmax, ssum, srecip)


def body_main(nc, tc, cpool, encpool, wpool, ppool, din,
              d_probs, d_newh, d_expw, d_rinv,
              transpose_f32, transpose_bf16,
              Wep_sb, Whp_sb, Wcs_sb, Whh_sb, Wout_sb, Wih_sb, emb_sb,
              vb, wv_sb, ic_sb, hid_sb, ones_sb,
              AT_sb, hT_sb, rinv_all, ctx_nat, ctxT_sb, indec_sb, rz_sb,
              n_sb, newhT_sb, newh_nat, lgT_sb, lg_nat, expv, probs_sb,
              smax, snegmax, ssum, srecip):
    sync = nc.sync
    # ---------------- stage A: hT, A^T, embedding ----------------
    for k in range(NK):
        p_tr = ppool.tile([128, 512], F32, tag="ptr", bufs=2, name=f"p_hT{k}")
        transpose_f32(p_tr[:, 0:B], hid_sb[:, 128 * k:128 * (k + 1)], B)
        nc.vector.tensor_copy(hT_sb[:, k, :], p_tr[:, 0:B])

    for j in range(NK):
        p_a = ppool.tile([128, B], F32, tag="ptr", bufs=2, name=f"p_A{j}")
        for k in range(NK):
            nc.tensor.matmul(p_a[:], Whp_sb[:, k, 128 * j:128 * (j + 1)],
                             hT_sb[:, k, :], start=(k == 0), stop=(k == NK - 1))
        nc.scalar.activation(AT_sb[:, j, :], p_a[:], AF.Identity,
                             bias=vb[:, VB_AB + j:VB_AB + j + 1])

    # argmax(in_char) -> one-hot -> emb rows into in_dec^T chunk 0
    icmax = cpool.tile([B, 1], F32)
    onehot = cpool.tile([B, V], F32)
    onehotT = cpool.tile([V, B], F32)
    nc.vector.tensor_reduce(icmax[:], ic_sb[:], axis=AX.X, op=ALU.max)
    nc.vector.tensor_scalar(onehot[:], ic_sb[:], icmax[:], None, op0=ALU.is_ge)
    p_oh = ppool.tile([128, B], F32, tag="ptr", bufs=2, name="p_oh")
    transpose_f32(p_oh[0:V, :], onehot[:], B)
    nc.vector.tensor_copy(onehotT[:], p_oh[0:V, :])
    p_et = ppool.tile([128, B], F32, tag="ptr", bufs=2, name="p_et")
    nc.tensor.matmul(p_et[0:EMB, :], emb_sb[:], onehotT[:], start=True, stop=True)
    nc.scalar.copy(indec_sb[0:EMB, 0, :], p_et[0:EMB, :])

    # ---------------- stage B: per-quad attention pipeline ----------------
    for q in range(B // 4):
        enc_nats = []
        expw_p0s = []
        for bi in range(4):
            b = 4 * q + bi
            # natural enc_b in bf16 (cast during SWDGE DMA):
            # [128, 5, 512]; row p of chunk j is t = 128j + p
            enc_nat = encpool.tile([128, len(TCH), H], BF16, tag="enc_nat",
                                   name=f"enc_nat{b}")
            enc_nats.append(enc_nat)
            nc.gpsimd.dma_start(
                enc_nat[:, 0:4, :],
                din["enc"][0:512, b, :].rearrange("(k p) c -> p k c", p=128))
            nc.gpsimd.dma_start(enc_nat[0:88, 4, :], din["enc"][512:600, b, :])

            # enc^T via bf16 PE transposes; 5 t-blocks share one psum bank
            encT = wpool.tile([128, NK, T], BF16, tag="encT", name=f"encT{b}")
            for k in range(NK):
                p_tr = ppool.tile([128, T], BF16, tag="ptr", bufs=2,
                                  name=f"p_tr{b}_{k}")
                for (ti, (t0, tw)) in enumerate(TCH):
                    transpose_bf16(p_tr[:, t0:t0 + tw],
                                   enc_nat[0:tw, ti, 128 * k:128 * (k + 1)], tw)
                if k % 2 == 0:
                    nc.vector.tensor_copy(encT[:, k, :], p_tr[:])
                else:
                    nc.scalar.copy(encT[:, k, :], p_tr[:])

            # enc_attn^T[j] = W_ep[:,j].T @ enc^T ; then tanh(+A bias)
            # psum [128, 1024]: halves at 0 and 512 so each matmul stays
            # in-bank while tanh reads the contiguous [0:600] span
            tanhT = wpool.tile([128, NK, T], BF16, tag="tanhT", name=f"tanhT{b}")
            for j in range(NK):
                p_m = ppool.tile([128, 1024], F32, tag="pmain", bufs=2,
                                 name=f"p_m{b}_{j}")
                for (o0, t0, tw) in ((0, 0, 512), (512, 512, 88)):
                    for k in range(NK):
                        nc.tensor.matmul(
                            p_m[:, o0:o0 + tw],
                            Wep_sb[:, k, 128 * j:128 * (j + 1)],
                            encT[:, k, t0:t0 + tw],
                            start=(k == 0), stop=(k == NK - 1))
                nc.scalar.activation(tanhT[:, j, :], p_m[:, 0:600], AF.Tanh,
                                     bias=AT_sb[:, j, b:b + 1])

            # scores row (partition 0): w_v^T @ tanh^T
            qscA = ppool.tile([1, 512], F32, tag="qrow", bufs=2, name=f"qscA{b}")
            qscB = ppool.tile([1, 512], F32, tag="qrow", bufs=2, name=f"qscB{b}")
            for (qsc, t0) in ((qscA, 0), (qscB, 300)):
                for j in range(NK):
                    nc.tensor.matmul(
                        qsc[0:1, 0:300],
                        wv_sb[:, j:j + 1],
                        tanhT[:, j, t0:t0 + 300],
                        start=(j == 0), stop=(j == NK - 1))

            # softmax pieces at partition 0
            # sred cols: 0,1 = halves' max, 2 = row max, 3 = -max
            sred = wpool.tile([1, 4], F32, tag="sred", name=f"sred{b}")
            nc.vector.tensor_reduce(sred[0:1, 0:1], qscA[0:1, 0:300],
                                    axis=AX.X, op=ALU.max)
            nc.vector.tensor_reduce(sred[0:1, 1:2], qscB[0:1, 0:300],
                                    axis=AX.X, op=ALU.max)
            nc.vector.tensor_reduce(sred[0:1, 2:3], sred[0:1, 0:2],
                                    axis=AX.X, op=ALU.max)
            nc.vector.tensor_scalar_mul(sred[0:1, 3:4], sred[0:1, 2:3], -1.0)
            expw_p0 = wpool.tile([1, T], F32, tag="expw_p0", bufs=3,
                                 name=f"expw_p0{b}")
            expw_p0s.append(expw_p0)
            ssum2 = wpool.tile([1, 2], F32, tag="ssum2", name=f"ssum2{b}")
            nc.scalar.activation(expw_p0[0:1, 0:300], qscA[0:1, 0:300], AF.Exp,
                                 bias=sred[0:1, 3:4], accum_out=ssum2[0:1, 0:1])
            nc.scalar.activation(expw_p0[0:1, 300:600], qscB[0:1, 0:300], AF.Exp,
                                 bias=sred[0:1, 3:4], accum_out=ssum2[0:1, 1:2])
            stot = wpool.tile([1, 1], F32, tag="stot", name=f"stot{b}")
            nc.vector.tensor_reduce(stot[0:1, :], ssum2[0:1, 0:2],
                                    axis=AX.X, op=ALU.add)
            nc.vector.reciprocal(rinv_all[0:1, b:b + 1], stot[0:1, :])
            # unnormalised attention row -> DRAM (host divides by the sum)
            nc.scalar.dma_start(d_expw[b:b + 1, :], expw_p0[0:1, :])

        # attnw^T columns via ones outer-product (partition 0 only)
        awT = wpool.tile([128, len(TCH), 4], BF16, tag="awT", name=f"awT{q}")
        p_awt = ppool.tile([128, 32], F32, tag="ptr", bufs=2, name=f"p_awt{q}")
        for bi in range(4):
            for (ti, (t0, tw)) in enumerate(TCH):
                nc.tensor.matmul(p_awt[0:tw, 4 * ti + bi:4 * ti + bi + 1],
                                 expw_p0s[bi][0:1, t0:t0 + tw],
                                 ones_sb[0:1, 0:1],
                                 start=True, stop=True)
        for (ti, (t0, tw)) in enumerate(TCH):
            nc.vector.tensor_copy(awT[0:tw, ti, :],
                                  p_awt[0:tw, 4 * ti:4 * ti + 4])

        # context rows: ctx[b] = rinv * sum_t expw[b,t] * enc[t,b,:]
        for bi in range(4):
            b = 4 * q + bi
            pctx = ppool.tile([1, 512], F32, tag="qrow", bufs=2, name=f"pctx{b}")
            for (ti, (t0, tw)) in enumerate(TCH):
                nc.tensor.matmul(pctx[0:1, :],
                                 awT[0:tw, ti, bi:bi + 1],
                                 enc_nats[bi][0:tw, ti, :],
                                 start=(ti == 0), stop=(ti == len(TCH) - 1))
            ctx_p0 = wpool.tile([1, H], F32, tag="ctx_p0", bufs=2,
                                name=f"ctx_p0{b}")
            nc.scalar.activation(ctx_p0[0:1, :], pctx[0:1, :],
                                 AF.Copy, scale=rinv_all[0:1, b:b + 1])
            # move the row into place (DMA crosses partitions freely)
            nc.scalar.dma_start(ctx_nat[b:b + 1, :], ctx_p0[0:1, :])

    # ---------------- stage C: epilogue ----------------
    for k in range(NK):
        p_tr = ppool.tile([128, 512], F32, tag="ptr", bufs=2, name=f"p_ctxT{k}")
        transpose_f32(p_tr[:, 0:B], ctx_nat[:, 128 * k:128 * (k + 1)], B)
        nc.vector.tensor_copy(ctxT_sb[:, k, :], p_tr[:, 0:B])

    # context shrink into in_dec^T chunks 1,2
    for (cs0, n_rows, ci) in CS_SPLIT:
        p_cs = ppool.tile([128, B], F32, tag="ptr", bufs=2, name=f"p_cs{ci}")
        for k in range(NK):
            nc.tensor.matmul(p_cs[0:n_rows, :],
                             Wcs_sb[:, k, cs0:cs0 + n_rows],
                             ctxT_sb[:, k, :],
                             start=(k == 0), stop=(k == NK - 1))
        nc.scalar.activation(indec_sb[0:n_rows, ci, :],
                             p_cs[0:n_rows, :], AF.Identity,
                             bias=vb[0:n_rows, VB_CS + ci - 1:VB_CS + ci])

    # GRU gates, chunks of 128 over 3H
    def gate_matmuls(p_g, c, with_ih, with_hh):
        nmm = (3 if with_ih else 0) + (NK if with_hh else 0)
        i = 0
        if with_ih:
            for k in range(3):
                kw = GIK[k]
                nc.tensor.matmul(p_g[:], Wih_sb[0:kw, k, 128 * c:128 * (c + 1)],
                                 indec_sb[0:kw, k, :],
                                 start=(i == 0), stop=(i == nmm - 1))
                i += 1
        if with_hh:
            for k in range(NK):
                nc.tensor.matmul(p_g[:], Whh_sb[:, k, 128 * c:128 * (c + 1)],
                                 hT_sb[:, k, :],
                                 start=(i == 0), stop=(i == nmm - 1))
                i += 1

    for c in range(8):  # r and z gates
        p_g = ppool.tile([128, B], F32, tag="ptr", bufs=2, name=f"p_g{c}")
        gate_matmuls(p_g, c, True, True)
        nc.scalar.activation(rz_sb[:, c, :], p_g[:], AF.Sigmoid,
                             bias=vb[:, VB_RZ + c:VB_RZ + c + 1])

    hn_sb = cpool.tile([128, B], F32)
    rhn_sb = cpool.tile([128, B], F32)
    gin_sb = cpool.tile([128, B], F32)
    for k in range(NK):  # n gate chunks + new_h
        c = 8 + k
        p_gh = ppool.tile([128, B], F32, tag="ptr", bufs=2, name=f"p_gh{k}")
        gate_matmuls(p_gh, c, False, True)
        nc.scalar.activation(hn_sb[:], p_gh[:], AF.Identity,
                             bias=vb[:, VB_HN + k:VB_HN + k + 1])
        nc.vector.tensor_mul(rhn_sb[:], rz_sb[:, k, :], hn_sb[:])
        p_gi = ppool.tile([128, B], F32, tag="pmain", bufs=2, name=f"p_gi{k}")
        gate_matmuls(p_gi, c, True, False)
        nc.vector.tensor_add(gin_sb[:], p_gi[:, 0:B], rhn_sb[:])
        nc.scalar.activation(n_sb[:, k, :], gin_sb[:], AF.Tanh,
                             bias=vb[:, VB_IN + k:VB_IN + k + 1])
        # new_h = n + z*(h - n)
        nc.vector.tensor_sub(rhn_sb[:], hT_sb[:, k, :], n_sb[:, k, :])
        nc.vector.tensor_mul(rhn_sb[:], rz_sb[:, 4 + k, :], rhn_sb[:])
        nc.vector.tensor_add(newhT_sb[:, k, :], n_sb[:, k, :], rhn_sb[:])

    # new_h natural + DMA
    for k in range(NK):
        p_tr = ppool.tile([128, 512], F32, tag="ptr", bufs=2, name=f"p_nh{k}")
        transpose_f32(p_tr[0:B, 0:128], newhT_sb[:, k, :], 128)
        nc.vector.tensor_copy(newh_nat[:, 128 * k:128 * (k + 1)], p_tr[0:B, 0:128])
    sync.dma_start(d_newh.ap(), newh_nat[:])

    # classifier + softmax
    p_lg = ppool.tile([128, B], F32, tag="ptr", bufs=2, name="p_lg")
    for k in range(NK):
        nc.tensor.matmul(p_lg[0:V, :], Wout_sb[:, k, :], newhT_sb[:, k, :],
                         start=(k == 0), stop=(k == NK - 1))
    nc.scalar.activation(lgT_sb[:], p_lg[0:V, :], AF.Identity,
                         bias=vb[0:V, VB_BO:VB_BO + 1])
    p_lgn = ppool.tile([128, 512], F32, tag="ptr", bufs=2, name="p_lgn")
    transpose_f32(p_lgn[0:B, 0:V], lgT_sb[:], V)
    nc.vector.tensor_copy(lg_nat[:], p_lgn[0:B, 0:V])

    nc.vector.tensor_reduce(smax[:], lg_nat[:], axis=AX.X, op=ALU.max)
    nc.vector.tensor_scalar_mul(snegmax[:], smax[:], -1.0)
    nc.scalar.activation(expv[:], lg_nat[:], AF.Exp, bias=snegmax[:],
                         accum_out=ssum[:])
    nc.vector.reciprocal(srecip[:], ssum[:])
    nc.vector.tensor_scalar(probs_sb[:], expv[:], srecip[:], None, op0=ALU.mult)
    sync.dma_start(d_probs.ap(), probs_sb[:])
    sync.dma_start(d_rinv.ap(), rinv_all[:])


_CACHED = None


def _get_program():
    global _CACHED
    if _CACHED is None:
        _CACHED = build_program()
    return _CACHED


def make_in_maps(inputs):
    inp = {k: np.ascontiguousarray(np.asarray(v, dtype=np.float32))
           for k, v in inputs.items()}
    vecbank = np.zeros((128, VB_N), np.float32)
    wv_bank = np.zeros((128, NK), ml_dtypes.bfloat16)
    wv = inp["w_v"].reshape(H)
    ab = inp["b_hp"] + inp["b_ep"]
    brz = (inp["b_ih"] + inp["b_hh"])[0:2 * H]
    bin_ = inp["b_ih"][2 * H:]
    bhn = inp["b_hh"][2 * H:]
    for j in range(NK):
        wv_bank[:, j] = wv[128 * j:128 * (j + 1)]
        vecbank[:, VB_AB + j] = ab[128 * j:128 * (j + 1)]
        vecbank[:, VB_IN + j] = bin_[128 * j:128 * (j + 1)]
        vecbank[:, VB_HN + j] = bhn[128 * j:128 * (j + 1)]
    for c in range(8):
        vecbank[:, VB_RZ + c] = brz[128 * c:128 * (c + 1)]
    bcs = inp["b_cs"]
    vecbank[0:128, VB_CS + 0] = bcs[0:128]
    vecbank[0:CS - 128, VB_CS + 1] = bcs[128:CS]
    vecbank[0:V, VB_BO] = inp["b_out"]

    shared = {
        "W_hp": inp["W_hp"],
        "W_ep": inp["W_ep"].astype(ml_dtypes.bfloat16), "W_cs": inp["W_cs"],
        "emb": inp["emb"], "W_ih": inp["W_ih"], "W_hh": inp["W_hh"],
        "W_out": inp["W_out"], "vecbank": vecbank, "wv_bank": wv_bank,
    }
    in_maps = []
    for c in range(N_CORES):
        sl = slice(B * c, B * (c + 1))
        m = dict(shared)
        m["in_char"] = np.ascontiguousarray(inp["in_char"][sl])
        m["hidden"] = np.ascontiguousarray(inp["hidden"][0, sl])
        m["enc"] = np.ascontiguousarray(inp["encoder_output"][:, sl])
        in_maps.append(m)
    return in_maps


def finish_outputs(core_results):
    probs = np.concatenate([r["out_probs"] for r in core_results], axis=0)
    newh = np.concatenate([r["out_newh"] for r in core_results], axis=0)[None]
    aw = []
    for r in core_results:
        expw = r["out_expw"]                      # [B, T], unnormalised
        rinv = r["out_rinv"]                      # [1, B]
        aw.append(expw * rinv.reshape(B, 1))
    attnw = np.concatenate(aw, axis=0)
    return probs, newh, attnw


def run(inputs, trace=False):
    nc = _get_program()
    in_maps = make_in_maps(inputs)
    res = bass_utils.run_bass_kernel_spmd(
        nc, in_maps, core_ids=list(range(N_CORES)), trace=trace)
    return finish_outputs(res.results), res


def kernel(**inputs):
    out, _ = run(inputs)
    return out


# revision 16
# speedup vs baseline: 277.4372x; 1.1257x over previous
"""Trainium2 Bass kernel for a single-step attention GRU decoder.

Math (per batch row b):
    A        = hidden @ W_hp + b_hp + b_ep                  [B, H]
    enc_attn = enc[t,b,:] @ W_ep                            [T, B, H]
    scores   = tanh(enc_attn + A[b]) @ w_v                  [B, T]
    attnw    = softmax(scores, axis=t)                      [B, T]   (output 3)
    context  = sum_t attnw[b,t] * enc[t,b,:]                [B, H]
    cs       = context @ W_cs + b_cs                        [B, 250]
    in_dec   = concat(emb[argmax(in_char)], cs)             [B, 300]
    GRU(in_dec, hidden) -> new_h                            [B, H]   (output 2)
    probs    = softmax(new_h @ W_out + b_out)               [B, 83]  (output 1)

Sharding: data-parallel over B=256 across 8 cores (32 rows each); all
weights replicated.  Inside a core everything runs in a "transposed"
layout (feature dim on partitions, batch on the free dim) so weight
matrices are stationary matmul operands in their natural [K_in, M_out]
layout and per-feature biases are per-partition scalars fused into
ScalarE activations.

enc (39 MB/core) is DMAed in natural layout [t, h] per batch row,
transposed on the PE (128x128 identity matmuls) to get enc^T [h, t] for
the score matmul; the natural copy is reused as the moving operand of
the context matmul (stationary = attnw^T column, contraction over t).
The big score matmul runs in float32r (single-pass fp32, 1 cycle/row at
N>=256 vs 4 for full fp32).

Engine APs can only start at partitions {0,32,64,96}, so per-row work
is placed at those bases (4 batch rows per "quad"), single rows move
between partitions via SBUF<->SBUF DMA, and the attention softmax is
only computed up to exp() on device - the 1/sum normalisation of the
attnw output happens on the host while the context matmul folds the
normalisation into its PSUM->SBUF eviction (ScalarE scale).
"""

import ml_dtypes
import numpy as np

import concourse.bass as bass
import concourse.bacc as bacc
import concourse.mybir as mybir
import concourse.tile as tile
from concourse import bass_utils, masks

dt = mybir.dt
F32 = dt.float32
F32R = dt.float32r
BF16 = dt.bfloat16
AF = mybir.ActivationFunctionType
ALU = mybir.AluOpType
AX = mybir.AxisListType

N_CORES = 8
B = 32          # per-core batch
T = 600
H = 512
V = 83
EMB = 50
CS = 250        # 5*EMB
GI = 300        # 6*EMB
G3 = 3 * H      # 1536

TCH = [(0, 128), (128, 128), (256, 128), (384, 128), (512, 88)]
NK = H // 128   # 4 h chunks

# vecbank columns
VB_WV = 0       # 4: w_v chunks
VB_AB = 4       # 4: b_hp + b_ep chunks
VB_RZ = 8       # 8: (b_ih + b_hh)[0:1024] chunks
VB_IN = 16      # 4: b_ih[1024:1536] chunks
VB_HN = 20      # 4: b_hh[1024:1536] chunks
VB_CS = 24      # 2: b_cs chunks (128, 122)
VB_BO = 26      # 1: b_out (83)
VB_N = 27

# in_dec^T partition chunks: [emb(50) | cs 0:128 | cs 128:250]
GIK = [EMB, 128, CS - 128]          # 50, 128, 122
CS_SPLIT = [(0, 128, 1), (128, CS - 128, 2)]   # (cs_off, n_rows, chunk_idx)


def _r(ap):
    return ap.bitcast(F32R)


def build_program(loop_n=1):
    nc = bacc.Bacc("TRN2", target_bir_lowering=False, debug=False,
                   num_devices=N_CORES)

    din = {}
    def inp(name, shape):
        din[name] = nc.dram_tensor(name, list(shape), F32, kind="ExternalInput")
        return din[name]

    inp("in_char", (B, V))
    inp("hidden", (B, H))
    inp("enc", (T, B, H))
    inp("W_hp", (H, H))
    din["W_ep"] = nc.dram_tensor("W_ep", [H, H], BF16, kind="ExternalInput")
    din["wv_bank"] = nc.dram_tensor("wv_bank", [128, NK], BF16, kind="ExternalInput")
    inp("W_cs", (H, CS))
    inp("emb", (V, EMB))
    inp("W_ih", (GI, G3))        # device re-chunks rows to (50,128,122)
    inp("W_hh", (H, G3))
    inp("W_out", (H, V))
    inp("vecbank", (128, VB_N))

    d_probs = nc.dram_tensor("out_probs", [B, V], F32, kind="ExternalOutput")
    d_newh = nc.dram_tensor("out_newh", [B, H], F32, kind="ExternalOutput")
    d_expw = nc.dram_tensor("out_expw", [B, T], F32, kind="ExternalOutput")
    d_rinv = nc.dram_tensor("out_rinv", [1, B], F32, kind="ExternalOutput")

    with tile.TileContext(nc) as tc:
        with (
            tc.tile_pool(name="const", bufs=1) as cpool,
            tc.tile_pool(name="enc", bufs=6) as encpool,
            tc.tile_pool(name="work", bufs=2) as wpool,
            tc.tile_pool(name="psum", bufs=1, space=bass.MemorySpace.PSUM) as ppool,
        ):
            build_body(nc, tc, cpool, encpool, wpool, ppool, din,
                       d_probs, d_newh, d_expw, d_rinv, loop_n=loop_n)

    nc.compile()
    return nc


def build_body(nc, tc, cpool, encpool, wpool, ppool, din,
               d_probs, d_newh, d_expw, d_rinv, loop_n=1):
    sync = nc.sync

    # ---------------- constants into SBUF ----------------
    Wep_sb = cpool.tile([128, NK, H], BF16)
    sync.dma_start(Wep_sb[:], din["W_ep"].ap().rearrange("(k p) n -> p k n", p=128))
    Whp_sb = cpool.tile([128, NK, H], F32)
    sync.dma_start(Whp_sb[:], din["W_hp"].ap().rearrange("(k p) n -> p k n", p=128))
    Wcs_sb = cpool.tile([128, NK, CS], F32)
    sync.dma_start(Wcs_sb[:], din["W_cs"].ap().rearrange("(k p) n -> p k n", p=128))
    Whh_sb = cpool.tile([128, NK, G3], F32)
    sync.dma_start(Whh_sb[:], din["W_hh"].ap().rearrange("(k p) n -> p k n", p=128))
    Wout_sb = cpool.tile([128, NK, V], F32)
    sync.dma_start(Wout_sb[:], din["W_out"].ap().rearrange("(k p) n -> p k n", p=128))
    Wih_sb = cpool.tile([128, 3, G3], F32)
    sync.dma_start(Wih_sb[0:GIK[0], 0, :], din["W_ih"][0:50, :])
    sync.dma_start(Wih_sb[0:GIK[1], 1, :], din["W_ih"][50:178, :])
    sync.dma_start(Wih_sb[0:GIK[2], 2, :], din["W_ih"][178:300, :])
    emb_sb = cpool.tile([V, EMB], F32)
    sync.dma_start(emb_sb[:], din["emb"].ap())
    vb = cpool.tile([128, VB_N], F32)
    sync.dma_start(vb[:], din["vecbank"].ap())
    wv_sb = cpool.tile([128, NK], BF16)
    sync.dma_start(wv_sb[:], din["wv_bank"].ap())
    ic_sb = cpool.tile([B, V], F32)
    sync.dma_start(ic_sb[:], din["in_char"].ap())
    hid_sb = cpool.tile([B, H], F32)
    sync.dma_start(hid_sb[:], din["hidden"].ap())

    ident = cpool.tile([128, 128], F32)
    masks.make_identity(nc, ident[:])
    ident_bf = cpool.tile([128, 128], BF16)
    masks.make_identity(nc, ident_bf[:])
    ones_sb = cpool.tile([128, 1], F32)
    nc.gpsimd.memset(ones_sb[:], 1.0)

    # persistent per-core intermediates
    AT_sb = cpool.tile([128, NK, B], F32)        # (hid @ W_hp + b_hp + b_ep)^T
    hT_sb = cpool.tile([128, NK, B], F32)
    rinv_all = cpool.tile([1, B], F32)           # 1/sum(exp(scores)) per row
    ctx_nat = cpool.tile([B, H], F32)            # normalised context rows
    ctxT_sb = cpool.tile([128, NK, B], F32)
    indec_sb = cpool.tile([128, 3, B], F32)
    rz_sb = cpool.tile([128, 8, B], F32)         # r gates 0:4, z gates 4:8
    n_sb = cpool.tile([128, NK, B], F32)
    newhT_sb = cpool.tile([128, NK, B], F32)
    newh_nat = cpool.tile([B, H], F32)
    lgT_sb = cpool.tile([V, B], F32)
    lg_nat = cpool.tile([B, V], F32)
    expv = cpool.tile([B, V], F32)
    probs_sb = cpool.tile([B, V], F32)
    smax = cpool.tile([B, 1], F32)
    snegmax = cpool.tile([B, 1], F32)
    ssum = cpool.tile([B, 1], F32)
    srecip = cpool.tile([B, 1], F32)

    def transpose_f32(psum_out, in_ap, n_rows):
        # psum_out <- in_ap.T ; in_ap is [n_rows, M] at partition base 0
        nc.tensor.transpose(psum_out, in_ap, ident[0:n_rows, 0:n_rows])

    def transpose_bf16(psum_out, in_ap, n_rows):
        nc.tensor.transpose(psum_out, in_ap, ident_bf[0:n_rows, 0:n_rows])

    import contextlib
    loop_ctx = (tc.For_i(0, loop_n, 1) if loop_n > 1
                else contextlib.nullcontext())
    with loop_ctx:
        body_main(nc, tc, cpool, encpool, wpool, ppool, din,
                  d_probs, d_newh, d_expw, d_rinv,
                  transpose_f32, transpose_bf16,
                  Wep_sb, Whp_sb, Wcs_sb, Whh_sb, Wout_sb, Wih_sb, emb_sb,
                  vb, wv_sb, ic_sb, hid_sb, ones_sb,
                  AT_sb, hT_sb, rinv_all, ctx_nat, ctxT_sb, indec_sb, rz_sb,
                  n_sb, newhT_sb, newh_nat, lgT_sb, lg_nat, expv, probs_sb,
                  smax, snegmax, ssum, srecip)


def body_main(nc, tc, cpool, encpool, wpool, ppool, din,
              d_probs, d_newh, d_expw, d_rinv,
              transpose_f32, transpose_bf16,
              Wep_sb, Whp_sb, Wcs_sb, Whh_sb, Wout_sb, Wih_sb, emb_sb,
              vb, wv_sb, ic_sb, hid_sb, ones_sb,
              AT_sb, hT_sb, rinv_all, ctx_nat, ctxT_sb, indec_sb, rz_sb,
              n_sb, newhT_sb, newh_nat, lgT_sb, lg_nat, expv, probs_sb,
              smax, snegmax, ssum, srecip):
    sync = nc.sync
    # ---------------- stage A: hT, A^T, embedding ----------------
    for k in range(NK):
        p_tr = ppool.tile([128, 512], F32, tag="ptr", bufs=2, name=f"p_hT{k}")
        transpose_f32(p_tr[:, 0:B], hid_sb[:, 128 * k:128 * (k + 1)], B)
        nc.vector.tensor_copy(hT_sb[:, k, :], p_tr[:, 0:B])

    for j in range(NK):
        p_a = ppool.tile([128, B], F32, tag="ptr", bufs=2, name=f"p_A{j}")
        for k in range(NK):
            nc.tensor.matmul(p_a[:], Whp_sb[:, k, 128 * j:128 * (j + 1)],
                             hT_sb[:, k, :], start=(k == 0), stop=(k == NK - 1))
        nc.scalar.activation(AT_sb[:, j, :], p_a[:], AF.Identity,
                             bias=vb[:, VB_AB + j:VB_AB + j + 1])

    # argmax(in_char) -> one-hot -> emb rows into in_dec^T chunk 0
    icmax = cpool.tile([B, 1], F32)
    onehot = cpool.tile([B, V], F32)
    onehotT = cpool.tile([V, B], F32)
    nc.vector.tensor_reduce(icmax[:], ic_sb[:], axis=AX.X, op=ALU.max)
    nc.vector.tensor_scalar(onehot[:], ic_sb[:], icmax[:], None, op0=ALU.is_ge)
    p_oh = ppool.tile([128, B], F32, tag="ptr", bufs=2, name="p_oh")
    transpose_f32(p_oh[0:V, :], onehot[:], B)
    nc.vector.tensor_copy(onehotT[:], p_oh[0:V, :])
    p_et = ppool.tile([128, B], F32, tag="ptr", bufs=2, name="p_et")
    nc.tensor.matmul(p_et[0:EMB, :], emb_sb[:], onehotT[:], start=True, stop=True)
    nc.scalar.copy(indec_sb[0:EMB, 0, :], p_et[0:EMB, :])

    # ---------------- stage B: per-quad attention pipeline ----------------
    for q in range(B // 4):
        enc_nats = []
        expw_p0s = []
        for bi in range(4):
            b = 4 * q + bi
            # natural enc_b in bf16 (cast during SWDGE DMA):
            # [128, 5, 512]; row p of chunk j is t = 128j + p
            enc_nat = encpool.tile([128, len(TCH), H], BF16, tag="enc_nat",
                                   name=f"enc_nat{b}")
            enc_nats.append(enc_nat)
            nc.gpsimd.dma_start(
                enc_nat[:, 0:4, :],
                din["enc"][0:512, b, :].rearrange("(k p) c -> p k c", p=128))
            nc.gpsimd.dma_start(enc_nat[0:88, 4, :], din["enc"][512:600, b, :])

            # enc^T via bf16 PE transposes; 5 t-blocks share one psum bank
            encT = wpool.tile([128, NK, T], BF16, tag="encT", bufs=3, name=f"encT{b}")
            for k in range(NK):
                p_tr = ppool.tile([128, T], BF16, tag="ptr", bufs=2,
                                  name=f"p_tr{b}_{k}")
                for (ti, (t0, tw)) in enumerate(TCH):
                    transpose_bf16(p_tr[:, t0:t0 + tw],
                                   enc_nat[0:tw, ti, 128 * k:128 * (k + 1)], tw)
                if k % 4 != 3:
                    nc.vector.tensor_copy(encT[:, k, :], p_tr[:])
                else:
                    nc.scalar.copy(encT[:, k, :], p_tr[:])

            # enc_attn^T[j] = W_ep[:,j].T @ enc^T ; then tanh(+A bias)
            # psum [128, 1024]: halves at 0 and 512 so each matmul stays
            # in-bank while tanh reads the contiguous [0:600] span
            tanhT = wpool.tile([128, NK, T], BF16, tag="tanhT", bufs=3, name=f"tanhT{b}")
            for j in range(NK):
                p_m = ppool.tile([128, 1024], F32, tag="pmain", bufs=2,
                                 name=f"p_m{b}_{j}")
                for (o0, t0, tw) in ((0, 0, 512), (512, 512, 88)):
                    for k in range(NK):
                        nc.tensor.matmul(
                            p_m[:, o0:o0 + tw],
                            Wep_sb[:, k, 128 * j:128 * (j + 1)],
                            encT[:, k, t0:t0 + tw],
                            start=(k == 0), stop=(k == NK - 1))
                nc.scalar.activation(tanhT[:, j, :], p_m[:, 0:600], AF.Tanh,
                                     bias=AT_sb[:, j, b:b + 1])

            # scores row (partition 0): w_v^T @ tanh^T
            qscA = ppool.tile([1, 512], F32, tag="qrow", bufs=2, name=f"qscA{b}")
            qscB = ppool.tile([1, 512], F32, tag="qrow", bufs=2, name=f"qscB{b}")
            for (qsc, t0) in ((qscA, 0), (qscB, 300)):
                for j in range(NK):
                    nc.tensor.matmul(
                        qsc[0:1, 0:300],
                        wv_sb[:, j:j + 1],
                        tanhT[:, j, t0:t0 + 300],
                        start=(j == 0), stop=(j == NK - 1))

            # softmax pieces at partition 0
            # sred cols: 0,1 = halves' max, 2 = row max, 3 = -max
            sred = wpool.tile([1, 4], F32, tag="sred", name=f"sred{b}")
            nc.vector.tensor_reduce(sred[0:1, 0:1], qscA[0:1, 0:300],
                                    axis=AX.X, op=ALU.max)
            nc.vector.tensor_reduce(sred[0:1, 1:2], qscB[0:1, 0:300],
                                    axis=AX.X, op=ALU.max)
            nc.vector.tensor_reduce(sred[0:1, 2:3], sred[0:1, 0:2],
                                    axis=AX.X, op=ALU.max)
            nc.vector.tensor_scalar_mul(sred[0:1, 3:4], sred[0:1, 2:3], -1.0)
            expw_p0 = wpool.tile([1, T], F32, tag="expw_p0", bufs=3,
                                 name=f"expw_p0{b}")
            expw_p0s.append(expw_p0)
            ssum2 = wpool.tile([1, 2], F32, tag="ssum2", name=f"ssum2{b}")
            nc.scalar.activation(expw_p0[0:1, 0:300], qscA[0:1, 0:300], AF.Exp,
                                 bias=sred[0:1, 3:4], accum_out=ssum2[0:1, 0:1])
            nc.scalar.activation(expw_p0[0:1, 300:600], qscB[0:1, 0:300], AF.Exp,
                                 bias=sred[0:1, 3:4], accum_out=ssum2[0:1, 1:2])
            stot = wpool.tile([1, 1], F32, tag="stot", name=f"stot{b}")
            nc.vector.tensor_reduce(stot[0:1, :], ssum2[0:1, 0:2],
                                    axis=AX.X, op=ALU.add)
            nc.vector.reciprocal(rinv_all[0:1, b:b + 1], stot[0:1, :])
            # unnormalised attention row -> DRAM (host divides by the sum)
            nc.scalar.dma_start(d_expw[b:b + 1, :], expw_p0[0:1, :])

        # attnw^T columns via ones outer-product (partition 0 only)
        awT = wpool.tile([128, len(TCH), 4], BF16, tag="awT", name=f"awT{q}")
        p_awt = ppool.tile([128, 32], F32, tag="ptr", bufs=2, name=f"p_awt{q}")
        for bi in range(4):
            for (ti, (t0, tw)) in enumerate(TCH):
                nc.tensor.matmul(p_awt[0:tw, 4 * ti + bi:4 * ti + bi + 1],
                                 expw_p0s[bi][0:1, t0:t0 + tw],
                                 ones_sb[0:1, 0:1],
                                 start=True, stop=True)
        for (ti, (t0, tw)) in enumerate(TCH):
            nc.vector.tensor_copy(awT[0:tw, ti, :],
                                  p_awt[0:tw, 4 * ti:4 * ti + 4])

        # context rows: ctx[b] = rinv * sum_t expw[b,t] * enc[t,b,:]
        for bi in range(4):
            b = 4 * q + bi
            pctx = ppool.tile([1, 512], F32, tag="qrow", bufs=2, name=f"pctx{b}")
            for (ti, (t0, tw)) in enumerate(TCH):
                nc.tensor.matmul(pctx[0:1, :],
                                 awT[0:tw, ti, bi:bi + 1],
                                 enc_nats[bi][0:tw, ti, :],
                                 start=(ti == 0), stop=(ti == len(TCH) - 1))
            ctx_p0 = wpool.tile([1, H], F32, tag="ctx_p0", bufs=2,
                                name=f"ctx_p0{b}")
            nc.vector.tensor_scalar(ctx_p0[0:1, :], pctx[0:1, :],
                                    rinv_all[0:1, b:b + 1], None, op0=ALU.mult)
            # move the row into place (DMA crosses partitions freely)
            nc.scalar.dma_start(ctx_nat[b:b + 1, :], ctx_p0[0:1, :])

    # ---------------- stage C: epilogue ----------------
    for k in range(NK):
        p_tr = ppool.tile([128, 512], F32, tag="ptr", bufs=2, name=f"p_ctxT{k}")
        transpose_f32(p_tr[:, 0:B], ctx_nat[:, 128 * k:128 * (k + 1)], B)
        nc.vector.tensor_copy(ctxT_sb[:, k, :], p_tr[:, 0:B])

    # context shrink into in_dec^T chunks 1,2
    for (cs0, n_rows, ci) in CS_SPLIT:
        p_cs = ppool.tile([128, B], F32, tag="ptr", bufs=2, name=f"p_cs{ci}")
        for k in range(NK):
            nc.tensor.matmul(p_cs[0:n_rows, :],
                             Wcs_sb[:, k, cs0:cs0 + n_rows],
                             ctxT_sb[:, k, :],
                             start=(k == 0), stop=(k == NK - 1))
        nc.scalar.activation(indec_sb[0:n_rows, ci, :],
                             p_cs[0:n_rows, :], AF.Identity,
                             bias=vb[0:n_rows, VB_CS + ci - 1:VB_CS + ci])

    # GRU gates, chunks of 128 over 3H
    def gate_matmuls(p_g, c, with_ih, with_hh):
        nmm = (3 if with_ih else 0) + (NK if with_hh else 0)
        i = 0
        if with_ih:
            for k in range(3):
                kw = GIK[k]
                nc.tensor.matmul(p_g[:], Wih_sb[0:kw, k, 128 * c:128 * (c + 1)],
                                 indec_sb[0:kw, k, :],
                                 start=(i == 0), stop=(i == nmm - 1))
                i += 1
        if with_hh:
            for k in range(NK):
                nc.tensor.matmul(p_g[:], Whh_sb[:, k, 128 * c:128 * (c + 1)],
                                 hT_sb[:, k, :],
                                 start=(i == 0), stop=(i == nmm - 1))
                i += 1

    for c in range(8):  # r and z gates
        p_g = ppool.tile([128, B], F32, tag="ptr", bufs=2, name=f"p_g{c}")
        gate_matmuls(p_g, c, True, True)
        nc.scalar.activation(rz_sb[:, c, :], p_g[:], AF.Sigmoid,
                             bias=vb[:, VB_RZ + c:VB_RZ + c + 1])

    hn_sb = cpool.tile([128, B], F32)
    rhn_sb = cpool.tile([128, B], F32)
    gin_sb = cpool.tile([128, B], F32)
    for k in range(NK):  # n gate chunks + new_h
        c = 8 + k
        p_gh = ppool.tile([128, B], F32, tag="ptr", bufs=2, name=f"p_gh{k}")
        gate_matmuls(p_gh, c, False, True)
        nc.scalar.activation(hn_sb[:], p_gh[:], AF.Identity,
                             bias=vb[:, VB_HN + k:VB_HN + k + 1])
        nc.vector.tensor_mul(rhn_sb[:], rz_sb[:, k, :], hn_sb[:])
        p_gi = ppool.tile([128, B], F32, tag="pmain", bufs=2, name=f"p_gi{k}")
        gate_matmuls(p_gi, c, True, False)
        nc.vector.tensor_add(gin_sb[:], p_gi[:, 0:B], rhn_sb[:])
        nc.scalar.activation(n_sb[:, k, :], gin_sb[:], AF.Tanh,
                             bias=vb[:, VB_IN + k:VB_IN + k + 1])
        # new_h = n + z*(h - n)
        nc.vector.tensor_sub(rhn_sb[:], hT_sb[:, k, :], n_sb[:, k, :])
        nc.vector.tensor_mul(rhn_sb[:], rz_sb[:, 4 + k, :], rhn_sb[:])
        nc.vector.tensor_add(newhT_sb[:, k, :], n_sb[:, k, :], rhn_sb[:])

    # new_h natural + DMA
    for k in range(NK):
        p_tr = ppool.tile([128, 512], F32, tag="ptr", bufs=2, name=f"p_nh{k}")
        transpose_f32(p_tr[0:B, 0:128], newhT_sb[:, k, :], 128)
        nc.vector.tensor_copy(newh_nat[:, 128 * k:128 * (k + 1)], p_tr[0:B, 0:128])
    sync.dma_start(d_newh.ap(), newh_nat[:])

    # classifier + softmax
    p_lg = ppool.tile([128, B], F32, tag="ptr", bufs=2, name="p_lg")
    for k in range(NK):
        nc.tensor.matmul(p_lg[0:V, :], Wout_sb[:, k, :], newhT_sb[:, k, :],
                         start=(k == 0), stop=(k == NK - 1))
    nc.scalar.activation(lgT_sb[:], p_lg[0:V, :], AF.Identity,
                         bias=vb[0:V, VB_BO:VB_BO + 1])
    p_lgn = ppool.tile([128, 512], F32, tag="ptr", bufs=2, name="p_lgn")
    transpose_f32(p_lgn[0:B, 0:V], lgT_sb[:], V)
    nc.vector.tensor_copy(lg_nat[:], p_lgn[0:B, 0:V])

    nc.vector.tensor_reduce(smax[:], lg_nat[:], axis=AX.X, op=ALU.max)
    nc.vector.tensor_scalar_mul(snegmax[:], smax[:], -1.0)
    nc.scalar.activation(expv[:], lg_nat[:], AF.Exp, bias=snegmax[:],
                         accum_out=ssum[:])
    nc.vector.reciprocal(srecip[:], ssum[:])
    nc.vector.tensor_scalar(probs_sb[:], expv[:], srecip[:], None, op0=ALU.mult)
    sync.dma_start(d_probs.ap(), probs_sb[:])
    sync.dma_start(d_rinv.ap(), rinv_all[:])


_CACHED = None


def _get_program():
    global _CACHED
    if _CACHED is None:
        _CACHED = build_program()
    return _CACHED


def make_in_maps(inputs):
    inp = {k: np.ascontiguousarray(np.asarray(v, dtype=np.float32))
           for k, v in inputs.items()}
    vecbank = np.zeros((128, VB_N), np.float32)
    wv_bank = np.zeros((128, NK), ml_dtypes.bfloat16)
    wv = inp["w_v"].reshape(H)
    ab = inp["b_hp"] + inp["b_ep"]
    brz = (inp["b_ih"] + inp["b_hh"])[0:2 * H]
    bin_ = inp["b_ih"][2 * H:]
    bhn = inp["b_hh"][2 * H:]
    for j in range(NK):
        wv_bank[:, j] = wv[128 * j:128 * (j + 1)]
        vecbank[:, VB_AB + j] = ab[128 * j:128 * (j + 1)]
        vecbank[:, VB_IN + j] = bin_[128 * j:128 * (j + 1)]
        vecbank[:, VB_HN + j] = bhn[128 * j:128 * (j + 1)]
    for c in range(8):
        vecbank[:, VB_RZ + c] = brz[128 * c:128 * (c + 1)]
    bcs = inp["b_cs"]
    vecbank[0:128, VB_CS + 0] = bcs[0:128]
    vecbank[0:CS - 128, VB_CS + 1] = bcs[128:CS]
    vecbank[0:V, VB_BO] = inp["b_out"]

    shared = {
        "W_hp": inp["W_hp"],
        "W_ep": inp["W_ep"].astype(ml_dtypes.bfloat16), "W_cs": inp["W_cs"],
        "emb": inp["emb"], "W_ih": inp["W_ih"], "W_hh": inp["W_hh"],
        "W_out": inp["W_out"], "vecbank": vecbank, "wv_bank": wv_bank,
    }
    in_maps = []
    for c in range(N_CORES):
        sl = slice(B * c, B * (c + 1))
        m = dict(shared)
        m["in_char"] = np.ascontiguousarray(inp["in_char"][sl])
        m["hidden"] = np.ascontiguousarray(inp["hidden"][0, sl])
        m["enc"] = np.ascontiguousarray(inp["encoder_output"][:, sl])
        in_maps.append(m)
    return in_maps


def finish_outputs(core_results):
    probs = np.concatenate([r["out_probs"] for r in core_results], axis=0)
    newh = np.concatenate([r["out_newh"] for r in core_results], axis=0)[None]
    aw = []
    for r in core_results:
        expw = r["out_expw"]                      # [B, T], unnormalised
        rinv = r["out_rinv"]                      # [1, B]
        aw.append(expw * rinv.reshape(B, 1))
    attnw = np.concatenate(aw, axis=0)
    return probs, newh, attnw


def run(inputs, trace=False):
    nc = _get_program()
    in_maps = make_in_maps(inputs)
    res = bass_utils.run_bass_kernel_spmd(
        nc, in_maps, core_ids=list(range(N_CORES)), trace=trace)
    return finish_outputs(res.results), res


def kernel(**inputs):
    out, _ = run(inputs)
    return out
